# revision 1
# baseline (speedup 1.0000x reference)
"""Trainium2 Bass kernel for causal multi-head attention block.

Module: qkv = x @ W_attn + b_attn; causal softmax((q k^T)/sqrt(C)); y = (attn @ v) @ W_proj + b_proj
Shapes (hardcoded): x [8, 1024, 768], W_attn [768, 2304], W_proj [768, 768], H=12, D=64.

Sharding: data parallel over batch — core b computes batch element b (B == n_cores == 8).
No collectives; host scatters x and gathers y.

Per-core dataflow (all matmul operands float32r = 4-byte fp32 at full PE rate):
  1. x [1024,768] -> PE-transpose -> xT [768,1024]          (6 chunks of [128,1024])
  2. QT,KT [c,1024] = (W_attn chunk).T @ xT   (qkv transposed; W stationary)
     V [1024, 780]  = xT.T @ W_attn_vcols    (natural layout, +1.0 column per head)
  3. per head h, per 512-query group g:
       scoresT[tk,tq] = KT_h_tile.T @ QT_h     (causal-tight: tq >= tk tiles only)
       expT = Exp(scoresT * 1/sqrt(768))       (ScalarE, PSUM->SBUF)
       diag 128x128 block *= upper-tri mask
       av[65, tq]  += [V_h | 1].T @ expT       (row 64 = sum of exp = softmax denom)
       yT_h = av[0:64] * broadcast(1/av[64])   (recip on DVE, broadcast via K=1 matmul)
  4. out[tq, c] = (yT chunk).T @ W_proj chunk  (+bias), DMA natural rows to DRAM
"""

import math

import numpy as np

import concourse.bass as bass
import concourse.mybir as mybir
from concourse import bacc, tile
from concourse.bass_utils import run_bass_kernel_spmd
from concourse.masks import make_identity, make_upper_triangular

N_CORES = 8
B, T, C = 8, 1024, 768
H, D = 12, 64
C3 = 3 * C
NT = T // 128          # 8 token tiles
NK = C // 128          # 6 contraction chunks
NG = T // 512          # 2 query groups of 512
SCALE = 1.0 / math.sqrt(C)

F32 = mybir.dt.float32
F32R = mybir.dt.float32r
MULT = mybir.AluOpType.mult
EXP = mybir.ActivationFunctionType.Exp


def _build_nc(has_battn: bool, has_bproj: bool, num_devices: int, repeat: int = 1):
    nc = bacc.Bacc(
        "TRN2", target_bir_lowering=False, debug=False, num_devices=num_devices
    )

    x_d = nc.dram_tensor("x", (T, C), F32R, kind="ExternalInput")
    wa_d = nc.dram_tensor("W_attn", (C, C3), F32R, kind="ExternalInput")
    wp_d = nc.dram_tensor("W_proj", (C, C), F32R, kind="ExternalInput")
    ba_d = bp_d = None
    if has_battn:
        ba_d = nc.dram_tensor("b_attn", (C3,), F32, kind="ExternalInput")
    if has_bproj:
        bp_d = nc.dram_tensor("b_proj", (C,), F32, kind="ExternalInput")
    y_d = nc.dram_tensor("y", (T, C), F32, kind="ExternalOutput")

    with tile.TileContext(nc) as tc:
        with (
            tc.tile_pool(name="sb", bufs=1) as sb,
            tc.tile_pool(name="ps", bufs=1, space="PSUM") as ps,
        ):
            for _ in range(repeat):
                _emit(nc, sb, ps, x_d, wa_d, wp_d, ba_d, bp_d, y_d)
    nc.compile()
    return nc


def _emit(nc, sb, ps, x_d, wa_d, wp_d, ba_d, bp_d, y_d):
    sb_bufs = dict(_SB_BUFS)
    if ba_d is not None or bp_d is not None:
        # bias tiles need a few KiB of SBUF; shrink elastic pools to fit
        sb_bufs["expt"] = 3
        sb_bufs["wv"] = 6

    def st(shape, tag, dtype=F32R, name=None):
        return sb.tile(list(shape), dtype, tag=tag, bufs=sb_bufs[tag],
                       name=name or tag)

    def pt(shape, tag, dtype=F32, name=None):
        return ps.tile(list(shape), dtype, tag=tag, bufs=_PS_BUFS[tag],
                       name=name or tag)

    # --- constants ---
    # memset/affine_select can't encode float32r, and the BIR verifier demands
    # f32r matmul operands come from f32r-rounding producers — so constants are
    # built in plain f32 and DVE-copied into f32r tiles.
    ident32 = st([128, 128], "ident32", dtype=F32)
    make_identity(nc, ident32[:])
    ident = st([128, 128], "ident")
    nc.vector.tensor_copy(ident[:], ident32[:])
    tri32 = st([128, 128], "tri32", dtype=F32)
    make_upper_triangular(nc, tri32[:], val=1.0, diag=True)  # tri[r,c]=1 iff c>=r
    tri = st([128, 128], "tri")
    nc.vector.tensor_copy(tri[:], tri32[:])
    ones32 = st([1, 128], "ones32", dtype=F32)
    nc.gpsimd.memset(ones32[:], 1.0)
    ones = st([1, 128], "ones")
    nc.vector.tensor_copy(ones[:], ones32[:])
    vcol32 = st([128, D], "vcol32", dtype=F32)
    nc.gpsimd.memset(vcol32[:], 1.0)

    bqk = []
    bv_row = bp_row = None
    if ba_d is not None:
        for m in range(12):  # Q,K outchan tiles 0..1535
            bt = st([128, 1], "bqk", dtype=F32)
            nc.sync.dma_start(bt[:], ba_d.ap()[m * 128 : (m + 1) * 128])
            bqk.append(bt)
        bv_row = st([1, C], "bvrow", dtype=F32)
        nc.sync.dma_start(bv_row[:], ba_d.ap()[2 * C : 3 * C])
    if bp_d is not None:
        bp_row = st([1, C], "bprow", dtype=F32)
        nc.sync.dma_start(bp_row[:], bp_d.ap())

    # --- persistent activations ---
    xT = [
        [st([128, 512], "xT", name=f"xT{i}h{hf}") for hf in range(2)]
        for i in range(NK)
    ]
    qT = [st([128, T], "qkT", name=f"qT{i}") for i in range(NK)]
    kT = [st([128, T], "qkT", name=f"kT{i}") for i in range(NK)]
    # V natural layout [t, 12*128]: head h occupies a contiguous 128-col block
    # [V_h (64) | ones (64)], so the AV lhsT is a plain [128,128] slice and the
    # matmul emits yraw in PSUM rows 0:64 and sumexp replicated in rows 64:128.
    vt = [st([128, H * 2 * D], "v", name=f"v{i}") for i in range(NT)]
    yT = [st([128, T], "yT", name=f"yT{i}") for i in range(NK)]

    # --- phase 0: load x, transpose to xT ---
    def load_wa(m):
        # all 6 k-chunks of W_attn column-block m in one 3-D-AP DMA:
        # dest [128, k*128+n] <- W_attn[k*128+p, m*128+n]
        wam = st([128, NK * 128], "wa", name=f"wa{m}")
        nc.sync.dma_start(
            wam[:],
            wa_d.ap()[:, m * 128 : (m + 1) * 128].rearrange(
                "(k p) n -> p k n", p=128
            ),
        )
        return [wam[:, k * 128 : (k + 1) * 128] for k in range(NK)]

    def phase0(ts_range):
        for t in ts_range:
            xin = st([128, C], "xin")
            nc.sync.dma_start(xin[:], x_d.ap()[t * 128 : (t + 1) * 128, :])
            for c in range(NK):
                ptt = pt([128, 128], "qkv", dtype=F32R)
                nc.tensor.transpose(
                    ptt[:], xin[:, c * 128 : (c + 1) * 128], ident[:]
                )
                nc.vector.tensor_copy(
                    xT[c][t // 4][:, (t % 4) * 128 : (t % 4 + 1) * 128], ptt[:]
                )

    phase0(range(0, NT))

    # --- phase 1a: QT / KT (chunk c covers heads 2c, 2c+1) ---
    def emit_qk_dst(dst, m, was=None):
        if True:
            if was is None:
                was = load_wa(m)
            for g in range(NG):
                acc = pt([128, 512], "qkv")
                for k in range(NK):
                    nc.tensor.matmul(
                        acc[:],
                        was[k],
                        xT[k][g][:],
                        start=(k == 0),
                        stop=(k == NK - 1),
                    )
                dst_ap = dst[:, g * 512 : (g + 1) * 512]
                if ba_d is not None:
                    nc.vector.tensor_scalar_add(dst_ap, acc[:], bqk[m][:])
                else:
                    nc.vector.tensor_copy(dst_ap, acc[:])

    # --- phase 1b: V natural [t, heads*(D+1)] ---
    def load_wv(co, w):
        wvs = []
        for k in range(NK):
            wv = st([128, w], "wv", name=f"wv{co}_{k}")
            nc.sync.dma_start(
                wv[:],
                wa_d.ap()[k * 128 : (k + 1) * 128, 2 * C + co : 2 * C + co + w],
            )
            wvs.append(wv)
        return wvs

    def emit_v(wvs0):
        for t in range(NT):
            nc.vector.tensor_copy(
                vt[t][:].rearrange("p (h e) -> p h e", e=2 * D)[:, :, D : 2 * D],
                vcol32[:].unsqueeze(1).broadcast_to([128, H, D]),
            )
        for co, w in ((0, 512), (512, 256)):
            wvs = wvs0 if co == 0 else load_wv(co, w)
            for t in range(NT):
                acc = pt([128, w], "qkv")
                for k in range(NK):
                    nc.tensor.matmul(
                        acc[:],
                        xT[k][t // 4][:, (t % 4) * 128 : (t % 4 + 1) * 128],
                        wvs[k][:],
                        start=(k == 0),
                        stop=(k == NK - 1) and ba_d is None,
                    )
                if ba_d is not None:
                    nc.tensor.matmul(
                        acc[:],
                        ones32[0:1, 0:128],
                        bv_row[0:1, co : co + w],
                        start=False,
                        stop=True,
                    )
                nc.scalar.copy(
                    vt[t][:].rearrange("p (h e) -> p h e", e=2 * D)[
                        :, co // D : (co + w) // D, 0:D
                    ],
                    acc[:].rearrange("p (h e) -> p h e", e=D),
                )

    # --- phase 2: attention for one head ---
    # finishers (normalization chains) are deferred one scores-group so the
    # PE never stalls on the DVE reciprocal latency
    pending_fin = []

    def emit_head(h):
        ch, off = h // 2, (h % 2) * 64
        for g in range(NG):
            jmax = 4 * g + 3
            av = pt([128, 512], "av")
            groups = []
            j = 0
            while j <= jmax:
                nw_j = 512 - max(j * 128 - g * 512, 0)
                if nw_j == 512 and j + 1 <= jmax:
                    groups.append((j, j + 1))
                    j += 2
                else:
                    groups.append((j,))
                    j += 1

            def make_av(grp, e, av):
                def run():
                    for idx, jj in enumerate(grp):
                        nw = 512 - max(jj * 128 - g * 512, 0)
                        ow = 512 - nw
                        co = idx * 512
                        if jj >= 4 * g:  # diagonal block: keep tq >= tk
                            nc.vector.tensor_tensor(
                                e[:, co : co + 128], e[:, co : co + 128], tri[:],
                                op=MULT,
                            )
                        nc.tensor.matmul(
                            av[:, ow : ow + nw],
                            vt[jj][:, h * 2 * D : (h + 1) * 2 * D],
                            e[:, co : co + nw],
                            start=(jj == 0),
                            stop=(jj == jmax),
                        )
                return run

            pend_av = None
            for i, grp in enumerate(groups):
                widths = [512 - max(jj * 128 - g * 512, 0) for jj in grp]
                w_tot = widths[0] if len(grp) == 1 else 512 + widths[1]
                sc = pt([128, w_tot], "sc")
                e = st([128, w_tot], "expt")
                for idx, jj in enumerate(grp):
                    nw = widths[idx]
                    ow = 512 - nw
                    base = g * 512 + ow
                    co = idx * 512
                    nc.tensor.matmul(
                        sc[:, co : co + nw],
                        kT[ch][off : off + 64, jj * 128 : (jj + 1) * 128],
                        qT[ch][off : off + 64, base : base + nw],
                        start=True,
                        stop=True,
                    )
                nc.scalar.activation(e[:], sc[:], EXP, scale=SCALE)
                if i == 2 and pending_fin:
                    pending_fin.pop(0)()
                if pend_av is not None:
                    pend_av()
                pend_av = make_av(grp, e, av)
            pend_av()

            def fin(av=av, ch=ch, off=off, g=g):
                rc = st([64, 512], "rc")
                with nc.allow_low_precision("float32r is 4-byte fp32"):
                    nc.vector.reciprocal(rc[:], av[D : 2 * D, :])
                nc.vector.tensor_tensor(
                    yT[ch][off : off + 64, g * 512 : (g + 1) * 512],
                    av[0:D, :],
                    rc[:],
                    op=MULT,
                )

            pending_fin.append(fin)

    # --- phase 3: output projection, natural layout out ---
    def load_wp():
        wps = []
        for k in range(NK):
            wp = st([128, C], "wp")
            nc.sync.dma_start(wp[:], wp_d.ap()[k * 128 : (k + 1) * 128, :])
            wps.append(wp)
        return wps

    def emit_proj(wps):
        for t in range(NT):
            osb = st([128, C], "osb", dtype=F32)
            for co, w in ((0, 512), (512, 256)):
                acc = pt([128, w], "qkv")
                for k in range(NK):
                    nc.tensor.matmul(
                        acc[:],
                        yT[k][:, t * 128 : (t + 1) * 128],
                        wps[k][:, co : co + w],
                        start=(k == 0),
                        stop=(k == NK - 1) and bp_d is None,
                    )
                if bp_d is not None:
                    nc.tensor.matmul(
                        acc[:],
                        ones32[0:1, 0:128],
                        bp_row[0:1, co : co + w],
                        start=False,
                        stop=True,
                    )
                nc.scalar.copy(osb[:, co : co + w], acc[:])
            nc.sync.dma_start(y_d.ap()[t * 128 : (t + 1) * 128, :], osb[:])

    emit_qk_dst(qT[0], 0)
    wvs0 = load_wv(0, 512)
    emit_qk_dst(kT[0], 0 + NK)
    emit_v(wvs0)
    wps = None
    for c in range(NK):
        if c + 1 >= NK:
            wps = load_wp()
        emit_head(2 * c)
        if c + 1 < NK:
            emit_qk_dst(qT[c + 1], c + 1)
        emit_head(2 * c + 1)
        if c + 1 < NK:
            emit_qk_dst(kT[c + 1], c + 1 + NK)
    while pending_fin:
        pending_fin.pop(0)()
    emit_proj(wps)


# pool buffer counts, patched onto tile_pool via tags at tile() time
_SB_BUFS = {
    "ident": 1, "ident32": 1, "tri": 1, "tri32": 1, "ones": 1, "ones32": 1, "vcol32": 1, "bqk": 12, "bvrow": 1, "bprow": 1,
    "xT": 12, "qkT": 8, "v": 8, "yT": 6,
    "xin": 2, "wa": 3, "wv": 8, "expt": 4, "rc": 2, "wp": 6, "osb": 2,
}
_PS_BUFS = {"qkv": 2, "sc": 2, "av": 2}


_NC_CACHE = {}


def _get_nc(has_battn, has_bproj, num_devices=N_CORES):
    key = (has_battn, has_bproj, num_devices)
    if key not in _NC_CACHE:
        _NC_CACHE[key] = _build_nc(has_battn, has_bproj, num_devices)
    return _NC_CACHE[key]


class _CachedSpmdRunner:
    """Jit the 8-core shard_map once; re-execute on every kernel() call.

    Mirrors concourse.bass2jax.run_bass_via_pjrt's multi-core path, minus the
    per-call retrace/recompile.
    """

    def __init__(self, nc, n_cores):
        import jax
        from jax.sharding import Mesh, PartitionSpec
        from jax.experimental.shard_map import shard_map
        from concourse import bass2jax

        bass2jax.install_neuronx_cc_hook()
        assert nc.dbg_addr is None
        self.n_cores = n_cores
        partition_name = (
            nc.partition_id_tensor.name if nc.partition_id_tensor else None
        )
        in_names, out_names, out_avals = [], [], []
        for alloc in nc.m.functions[0].allocations:
            if not isinstance(alloc, mybir.MemoryLocationSet):
                continue
            name = alloc.memorylocations[0].name
            if alloc.kind == "ExternalInput":
                if name != partition_name:
                    in_names.append(name)
            elif alloc.kind == "ExternalOutput":
                out_names.append(name)
                out_avals.append(
                    jax.core.ShapedArray(
                        tuple(alloc.tensor_shape), mybir.dt.np(alloc.dtype)
                    )
                )
        self.in_names, self.out_names, self.out_avals = (
            in_names, out_names, out_avals,
        )
        all_in_names = list(in_names) + out_names
        if partition_name is not None:
            all_in_names.append(partition_name)

        def _body(*args):
            operands = list(args)
            if partition_name is not None:
                operands.append(bass2jax.partition_id_tensor())
            return tuple(
                bass2jax._bass_exec_p.bind(
                    *operands,
                    out_avals=tuple(out_avals),
                    in_names=tuple(all_in_names),
                    out_names=tuple(out_names),
                    lowering_input_output_aliases=(),
                    sim_require_finite=True,
                    sim_require_nnan=True,
                    nc=nc,
                )
            )

        import numpy as _np

        devices = jax.devices()[:n_cores]
        mesh = Mesh(_np.asarray(devices), ("core",))
        n_io = len(in_names) + len(out_names)
        self.fn = jax.jit(
            shard_map(
                _body,
                mesh=mesh,
                in_specs=(PartitionSpec("core"),) * n_io,
                out_specs=(PartitionSpec("core"),) * len(out_names),
                check_rep=False,
            ),
            keep_unused=True,
        )
        self.sharding = jax.sharding.NamedSharding(mesh, PartitionSpec("core"))

    def run(self, in_maps):
        import jax

        args = []
        for name in self.in_names:
            cat = np.concatenate(
                [np.asarray(m[name]) for m in in_maps], axis=0
            )
            args.append(jax.device_put(cat, self.sharding))
        for av in self.out_avals:
            z = np.zeros((self.n_cores * av.shape[0], *av.shape[1:]), av.dtype)
            args.append(jax.device_put(z, self.sharding))
        outs = self.fn(*args)
        return [
            {
                name: np.asarray(outs[i]).reshape(
                    self.n_cores, *self.out_avals[i].shape
                )[c]
                for i, name in enumerate(self.out_names)
            }
            for c in range(self.n_cores)
        ]


_RUNNER_CACHE = {}


def kernel(x, W_attn, b_attn, W_proj, b_proj):
    x = np.ascontiguousarray(np.asarray(x, dtype=np.float32))
    W_attn = np.ascontiguousarray(np.asarray(W_attn, dtype=np.float32))
    W_proj = np.ascontiguousarray(np.asarray(W_proj, dtype=np.float32))
    b_attn = np.asarray(b_attn, dtype=np.float32)
    b_proj = np.asarray(b_proj, dtype=np.float32)
    has_battn = bool(np.any(b_attn != 0.0))
    has_bproj = bool(np.any(b_proj != 0.0))

    nc = _get_nc(has_battn, has_bproj)
    in_maps = []
    for b in range(N_CORES):
        m = {"x": x[b], "W_attn": W_attn, "W_proj": W_proj}
        if has_battn:
            m["b_attn"] = b_attn
        if has_bproj:
            m["b_proj"] = b_proj
        in_maps.append(m)

    key = (has_battn, has_bproj)
    try:
        if key not in _RUNNER_CACHE:
            _RUNNER_CACHE[key] = _CachedSpmdRunner(nc, N_CORES)
        results = _RUNNER_CACHE[key].run(in_maps)
    except Exception:
        results = run_bass_kernel_spmd(
            nc, in_maps, core_ids=list(range(N_CORES))
        ).results
    return np.stack([results[b]["y"] for b in range(N_CORES)]).astype(np.float32)



# revision 64
# speedup vs baseline: 1.3196x; 1.3196x over previous
"""Trainium2 Bass kernel for causal multi-head attention block.

Module: qkv = x @ W_attn + b_attn; causal softmax((q k^T)/sqrt(C)); y = (attn @ v) @ W_proj + b_proj
Shapes (hardcoded): x [8, 1024, 768], W_attn [768, 2304], W_proj [768, 768], H=12, D=64.

Sharding: data parallel over batch - core b computes batch element b (B == n_cores == 8).
No collectives; host scatters x and gathers y. Host pre-casts x/W to bf16.

Per-core dataflow (all matmul operands bf16, fp32 PSUM accumulate):
  1. xT [768,1024] via 6 DMA XBAR transposes (no PE/DVE involvement)
  2. QT,KT [c,1024] = (W_attn chunk).T @ xT   (W stationary, xT moving)
     V natural [tok, 12*(64+1)]: per head [V_h | 1.0] so the AV matmul emits
     the softmax denominator as the 65th output column.
  3. per head h, blocks (query-group g, key-tile j) packed first-fit-decreasing
     into full 512-word PSUM bins -> 5 exp instructions/head:
       scoresT[tk,tq] = KT_h.T @ QT_h   (causal-tight tiles only)
       e = Exp(scoresT/sqrt(768))       (ScalarE, PSUM->SBUF bf16)
       diag tiles *= upper-tri mask     (Pool)
       avT[tq, 65] += e_block.T @ [V_h|1]  (65 moving rows/tile-pair: ~2x
                                            fewer PE rows than 512-wide AV)
       Y[t, h*64:] = avT[:,0:64] * recip(avT[:,64])  (normalization fused
                                            into the PSUM->SBUF copy, DVE)
     Scores run two heads ahead of AV; QK-chunk matmuls interleave as PE
     filler so exp latency and fin pacing never stall the PE.
  4. Y -> yT via PE transposes; out[tq, c] = yT.T @ W_proj (+bias), DMA out.
"""

import math
from collections import deque

import numpy as np

import concourse.bass as bass
import concourse.mybir as mybir
from concourse import bacc, tile
from concourse.bass_utils import run_bass_kernel_spmd
from concourse.masks import make_identity, make_upper_triangular

N_CORES = 8
B, T, C = 8, 1024, 768
H, D = 12, 64
C3 = 3 * C
NT = T // 128          # 8 token tiles
NK = C // 128          # 6 contraction chunks
NG = T // 512          # 2 query groups of 512
SCALE = 1.0 / math.sqrt(C)

F32 = mybir.dt.float32
BF16 = mybir.dt.bfloat16
MULT = mybir.AluOpType.mult
EXP = mybir.ActivationFunctionType.Exp


def _head_plan():
    """Causal score blocks for one head, packed into PSUM tiles.

    Returns (tiles, ecol): tiles is a list of (blocks, tile_width) where each
    block is (g, j, nw, tq0, diag, sc_off, e_off); ecol[(t, j)] gives the
    column in the head's e tile holding scoresT[:, tq-tile t] for key-tile j.
    """
    blocks = []
    for g in range(NG):
        for j in range(4 * g + 4):
            nw = 512 - max(j * 128 - g * 512, 0)
            tq0 = g * 512 + (512 - nw)
            blocks.append([g, j, nw, tq0])
    # first-fit-decreasing into 512-wide bins (bins end up exactly full)
    blocks.sort(key=lambda b: -b[2])
    bins = []
    for b in blocks:
        for bn in bins:
            if sum(x[2] for x in bn) + b[2] <= 512:
                bn.append(b)
                break
        else:
            bins.append([b])
    # pair bins into [128,1024] PSUM tiles; assign sc/e offsets
    tiles = []
    e_base = 0
    for i in range(0, len(bins), 2):
        pair = bins[i : i + 2]
        tb, off = [], 0
        for bn in pair:
            for g, j, nw, tq0 in bn:
                diag = j * 128 == tq0
                tb.append((g, j, nw, tq0, diag, off, e_base + off))
                off += nw
        tiles.append((tb, off))
        e_base += off
    assert e_base == 4608
    ecol = {}
    for tb, _ in tiles:
        for g, j, nw, tq0, diag, sc_off, e_off in tb:
            for t in range(4 * g, 4 * g + 4):
                lo = t * 128
                if tq0 <= lo and lo + 128 <= tq0 + nw:
                    ecol[(t, j)] = e_off + lo - tq0
    for t in range(NT):
        for j in range(t + 1):
            assert (t, j) in ecol, (t, j)
    return tiles, ecol


_PLAN_TILES, _PLAN_ECOL = _head_plan()


def _build_nc(has_battn: bool, has_bproj: bool, num_devices: int, repeat: int = 1):
    nc = bacc.Bacc(
        "TRN2", target_bir_lowering=False, debug=False, num_devices=num_devices
    )

    x_d = nc.dram_tensor("x", (C, T), BF16, kind="ExternalInput")  # pre-transposed on host
    wa_d = nc.dram_tensor("W_attn", (C, C3), BF16, kind="ExternalInput")
    wp_d = nc.dram_tensor("W_proj", (C, C), BF16, kind="ExternalInput")
    ba_d = bp_d = None
    if has_battn:
        ba_d = nc.dram_tensor("b_attn", (C3,), F32, kind="ExternalInput")
    if has_bproj:
        bp_d = nc.dram_tensor("b_proj", (C,), F32, kind="ExternalInput")
    y_d = nc.dram_tensor("y", (T, C), F32, kind="ExternalOutput")

    with tile.TileContext(nc) as tc:
        with (
            tc.tile_pool(name="sb", bufs=1) as sb,
            tc.tile_pool(name="ps", bufs=1, space="PSUM") as ps,
        ):
            for _ in range(repeat):
                _emit(nc, sb, ps, x_d, wa_d, wp_d, ba_d, bp_d, y_d)
    nc.compile()
    return nc


def _emit(nc, sb, ps, x_d, wa_d, wp_d, ba_d, bp_d, y_d):
    def st(shape, tag, dtype=BF16, name=None):
        return sb.tile(list(shape), dtype, tag=tag, bufs=_SB_BUFS[tag],
                       name=name or tag)

    def pt(shape, tag, dtype=F32, name=None):
        return ps.tile(list(shape), dtype, tag=tag, bufs=_PS_BUFS[tag],
                       name=name or tag)

    # --- constants ---
    ident32 = st([128, 128], "ident32", dtype=F32)
    make_identity(nc, ident32[:])
    ident = st([128, 128], "ident")
    nc.vector.tensor_copy(ident[:], ident32[:])
    tri32 = st([128, 128], "tri32", dtype=F32)
    make_upper_triangular(nc, tri32[:], val=1.0, diag=True)  # tri[r,c]=1 iff c>=r
    tri = st([128, 128], "tri")
    nc.vector.tensor_copy(tri[:], tri32[:])

    bqk = []
    ones = bv_row = bp_row = None
    if ba_d is not None or bp_d is not None:
        ones32 = st([1, 128], "ones32", dtype=F32)
        nc.gpsimd.memset(ones32[:], 1.0)
        ones = st([1, 128], "ones")
        nc.vector.tensor_copy(ones[:], ones32[:])
    if ba_d is not None:
        for m in range(12):  # Q,K outchan tiles 0..1535
            bt = st([128, 1], "bqk", dtype=F32)
            nc.sync.dma_start(bt[:], ba_d.ap()[m * 128 : (m + 1) * 128])
            bqk.append(bt)
        bv32 = st([1, C], "bv32", dtype=F32)
        nc.sync.dma_start(bv32[:], ba_d.ap()[2 * C : 3 * C])
        bv_row = st([1, C], "bvrow")
        nc.vector.tensor_copy(bv_row[:], bv32[:])
    if bp_d is not None:
        bp32 = st([1, C], "bp32", dtype=F32)
        nc.sync.dma_start(bp32[:], bp_d.ap())
        bp_row = st([1, C], "bprow")
        nc.vector.tensor_copy(bp_row[:], bp32[:])

    # --- persistent activations ---
    qT = [st([128, T], "qkT", name=f"qT{c}") for c in range(NK)]
    kT = [st([128, T], "qkT", name=f"kT{c}") for c in range(NK)]
    vt = [st([128, H * 65], "v", name=f"v{t}") for t in range(NT)]
    Yn = [st([128, C], "ynat", name=f"Y{t}") for t in range(NT)]
    yTc = [st([128, T], "yT", name=f"yT{c}") for c in range(NK)]

    # --- DMAs (SP queue; emission order = issue order) ---
    was = []

    def load_wa(m):
        wam = st([128, NK * 128], "wa", name=f"wa{m}")
        nc.sync.dma_start(
            wam[:],
            wa_d.ap()[:, m * 128 : (m + 1) * 128].rearrange(
                "(k p) n -> p k n", p=128
            ),
        )
        return wam

    was.append(load_wa(0))      # Q chunk 0
    was.append(load_wa(6))      # K chunk 0
    xT = []
    for c in range(NK):
        xt = st([128, T], "xT", name=f"xT{c}")
        xT.append(xt)
        nc.sync.dma_start(xt[:], x_d.ap()[c * 128 : (c + 1) * 128, :])
    was.append(load_wa(1))
    was.append(load_wa(7))
    # all W_attn V columns, every k-chunk, in one 3-D-AP DMA
    wvall = st([128, NK * C], "wv")
    nc.sync.dma_start(
        wvall[:],
        wa_d.ap()[:, 2 * C : 3 * C].rearrange("(k p) n -> p k n", p=128),
    )
    for c in range(2, NK):
        was.append(load_wa(c))
        was.append(load_wa(c + 6))
    wa_of = {}
    order = [0, 6, 1, 7] + [m for c in range(2, NK) for m in (c, c + 6)]
    for i, m in enumerate(order):
        wa_of[m] = was[i]
    wpall = st([128, NK * C], "wp")
    nc.sync.dma_start(
        wpall[:], wp_d.ap().rearrange("(k p) n -> p k n", p=128)
    )

    # --- QK-chunk unit: thunk list (filler granularity = 1 matmul) ---
    def qk_unit_thunks(dst, m, cp_eng=None):
        ths = []
        for g in range(NG):
            box = {}

            def mk(k, g=g, box=box):
                def run():
                    if k == 0:
                        box["acc"] = pt([128, 512], "qkv")
                    nc.tensor.matmul(
                        box["acc"][:],
                        wa_of[m][:, k * 128 : (k + 1) * 128],
                        xT[k][:, g * 512 : (g + 1) * 512],
                        start=(k == 0),
                        stop=(k == NK - 1),
                    )
                return run

            ths += [mk(k) for k in range(NK)]

            def cp(g=g, box=box, m=m):
                dst_ap = dst[:, g * 512 : (g + 1) * 512]
                if ba_d is not None:
                    nc.vector.tensor_scalar_add(dst_ap, box["acc"][:], bqk[m][:])
                elif cp_eng == "act":
                    nc.scalar.copy(dst_ap, box["acc"][:])
                else:
                    nc.vector.tensor_copy(dst_ap, box["acc"][:])

            ths.append(cp)
        return ths

    # --- V natural [tok, 12*(64+1)] ---
    def emit_v(trange):
        for t in trange:
            nc.gpsimd.memset(vt[t][:], 1.0)  # ones cols; data cols overwritten
            for co, w in ((0, 512), (512, 256)):
                acc = pt([128, w], "qkv")
                for k in range(NK):
                    nc.tensor.matmul(
                        acc[:],
                        xT[k][:, t * 128 : (t + 1) * 128],
                        wvall[:, k * C + co : k * C + co + w],
                        start=(k == 0),
                        stop=(k == NK - 1) and ba_d is None,
                    )
                if ba_d is not None:
                    nc.tensor.matmul(
                        acc[:],
                        ones[0:1, 0:128],
                        bv_row[0:1, co : co + w],
                        start=False,
                        stop=True,
                    )
                nc.scalar.copy(
                    vt[t][:].rearrange("p (h e) -> p h e", e=65)[
                        :, co // D : (co + w) // D, 0:D
                    ],
                    acc[:].rearrange("p (h e) -> p h e", e=D),
                )

    # --- per-head scores + exp + mask ---
    e_of = {}

    def emit_sc(h):
        ch, off = h // 2, (h % 2) * 64
        e = st([128, 4608], "expt", name=f"e{h}")
        e_of[h] = e
        for tb, tw in _PLAN_TILES:
            sc = pt([128, 1024], "sc")
            for g, j, nw, tq0, diag, sc_off, e_off in tb:
                nc.tensor.matmul(
                    sc[:, sc_off : sc_off + nw],
                    kT[ch][off : off + 64, j * 128 : (j + 1) * 128],
                    qT[ch][off : off + 64, tq0 : tq0 + nw],
                    start=True,
                    stop=True,
                )
            e_base = tb[0][6]
            nc.scalar.activation(
                e[:, e_base : e_base + tw], sc[:, 0:tw], EXP, scale=SCALE
            )
            for g, j, nw, tq0, diag, sc_off, e_off in tb:
                if diag:
                    nc.gpsimd.tensor_tensor(
                        e[:, e_off : e_off + 128],
                        e[:, e_off : e_off + 128],
                        tri[:],
                        op=MULT,
                    )

    # --- per-head AV^T + fused-normalization fin, interleaved with filler ---
    def tp_pe(chh, t, tag="qkv"):
        ptt = pt([128, 128], tag, dtype=BF16)
        nc.tensor.transpose(
            ptt[:], Yn[t][:, chh * 128 : (chh + 1) * 128], ident[:]
        )
        nc.vector.tensor_copy(yTc[chh][:, t * 128 : (t + 1) * 128], ptt[:])

    def _avT_tile(h, t, e):
        av = pt([128, 65], "av")
        for j in range(t + 1):
            nc.tensor.matmul(
                av[:],
                e[:, _PLAN_ECOL[(t, j)] : _PLAN_ECOL[(t, j)] + 128],
                vt[j][:, h * 65 : (h + 1) * 65],
                start=(j == 0),
                stop=(j == t),
            )
        rc = st([128, 1], "rc", dtype=F32)
        nc.vector.reciprocal(rc[:], av[:, 64:65])
        nc.vector.tensor_scalar_mul(
            Yn[t][:, h * D : (h + 1) * D], av[:, 0:D], rc[:]
        )

    def emit_avT(h, filler, inline_tp=None, cb=None):
        e = e_of.pop(h)
        for t in range(NT):
            _avT_tile(h, t, e)
            if inline_tp is not None:
                tp_pe(inline_tp, t)
            if cb is not None:
                cb(t)
            filler.step(2 if t < 5 else 1)

    def emit_avT_pair(ha, hb, filler, inline_tp=(None, None)):
        ea, eb = e_of.pop(ha), e_of.pop(hb)
        for t in range(NT):
            _avT_tile(ha, t, ea)
            if inline_tp[0] is not None:
                tp_pe(inline_tp[0], t)
            filler.step(2 if t < 5 else 1)
            _avT_tile(hb, t, eb)
            if inline_tp[1] is not None:
                tp_pe(inline_tp[1], t)
            filler.step(2 if t < 5 else 1)

    class Filler:
        def __init__(self):
            self.q = deque()

        def add(self, ths, uid=10**9):
            self.q.extend((uid, th) for th in ths)

        def step(self, n=1):
            for _ in range(n):
                if self.q:
                    self.q.popleft()[1]()

        def ensure(self, uid):
            while self.q and self.q[0][0] <= uid:
                self.q.popleft()[1]()

        def drain(self):
            while self.q:
                self.q.popleft()[1]()

    # --- Y -> yT transpose via DMA XBAR (zero-PE thunks for early chunks) ---
    def tpdma_thunks(chs):
        ths = []
        for chh in chs:
            for t in range(NT):
                def run(chh=chh, t=t):
                    nc.sync.dma_start_transpose(
                        yTc[chh][:, t * 128 : (t + 1) * 128],
                        Yn[t][:, chh * 128 : (chh + 1) * 128],
                    )
                ths.append(run)
        return ths

    # --- output projection, split so proj fills the attention tail:
    # proj_partial(t) = co0 contraction k0..4 (needs only chunks 0..4);
    # proj_finish(t) = co0 k5 + full co1 + copies + stores (needs chunk 5,
    # i.e. fin(h10/h11, t) + the inline ch5 transpose).
    pacc = {}

    def proj_partial_thunks(t):
        ths = []

        def mk(k, t=t):
            def run():
                if k == 0:
                    pacc[t] = pt([128, 512], "qkv")
                nc.tensor.matmul(
                    pacc[t][:],
                    yTc[k][:, t * 128 : (t + 1) * 128],
                    wpall[:, k * C : k * C + 512],
                    start=(k == 0),
                    stop=False,
                )
            return run

        return [mk(k) for k in range(5)]

    def proj_finish(t):
        osb = st([128, C], "osb", dtype=F32)
        acc = pacc.pop(t)
        nc.tensor.matmul(
            acc[:],
            yTc[5][:, t * 128 : (t + 1) * 128],
            wpall[:, 5 * C : 5 * C + 512],
            start=False,
            stop=bp_d is None,
        )
        if bp_d is not None:
            nc.tensor.matmul(
                acc[:], ones[0:1, 0:128], bp_row[0:1, 0:512],
                start=False, stop=True,
            )
        nc.scalar.copy(osb[:, 0:512], acc[:])
        # store the big half as soon as its copy lands: only the small co1
        # strip transfers after the last compute
        nc.sync.dma_start(y_d.ap()[t * 128 : (t + 1) * 128, 0:512], osb[:, 0:512])
        acc1 = pt([128, 256], "sc")  # sc banks are free once exps are done
        for k in range(NK):
            nc.tensor.matmul(
                acc1[:],
                yTc[k][:, t * 128 : (t + 1) * 128],
                wpall[:, k * C + 512 : (k + 1) * C],
                start=(k == 0),
                stop=(k == NK - 1) and bp_d is None,
            )
        if bp_d is not None:
            nc.tensor.matmul(
                acc1[:], ones[0:1, 0:128], bp_row[0:1, 512:768],
                start=False, stop=True,
            )
        if t == NT - 1:
            nc.vector.tensor_copy(osb[:, 512:768], acc1[:])
        else:
            nc.scalar.copy(osb[:, 512:768], acc1[:])
        nc.sync.dma_start(
            y_d.ap()[t * 128 : (t + 1) * 128, 512:768], osb[:, 512:768]
        )

    # --- schedule ---
    # p-state warmup: the PE needs ~3us of continuous busy to reach max clock;
    # burn identity matmuls into the (otherwise idle) sc bank while the first
    # DMAs are in flight so real matmuls start at peak frequency.
    for i in range(37):
        wsc = pt([128, 1024], "sc") if i % 8 == 0 else wsc  # noqa: F821
        wid = ident32 if i < 3 else ident  # f32 rows are 4x slower: good early
        nc.tensor.matmul(
            wsc[:, (i % 8) * 128 : (i % 8 + 1) * 128],
            wid[:],
            wid[:],
            start=True,
            stop=True,
        )

    fill = Filler()
    # chunk 0 k-major: all four acc groups (qT0/kT0 x g0/g1) stream each x
    # chunk as it arrives; kT0 accs borrow the (still idle) av banks
    c0_accs = {}
    for k in range(NK):
        for dst, m, gg, tag in (
            (qT[0], 0, 0, "qkv"), (qT[0], 0, 1, "qkv"),
            (kT[0], 6, 0, "av"), (kT[0], 6, 1, "av"),
        ):
            key = (m, gg)
            if k == 0:
                c0_accs[key] = pt([128, 512], tag)
            nc.tensor.matmul(
                c0_accs[key][:],
                wa_of[m][:, k * 128 : (k + 1) * 128],
                xT[k][:, gg * 512 : (gg + 1) * 512],
                start=(k == 0),
                stop=(k == NK - 1),
            )
    for dst, m, gg in (
        (qT[0], 0, 0), (qT[0], 0, 1), (kT[0], 6, 0), (kT[0], 6, 1)
    ):
        dst_ap = dst[:, gg * 512 : (gg + 1) * 512]
        if ba_d is not None:
            nc.vector.tensor_scalar_add(dst_ap, c0_accs[(m, gg)][:], bqk[m][:])
        else:
            nc.scalar.copy(dst_ap, c0_accs[(m, gg)][:])
    emit_sc(0)
    for th in qk_unit_thunks(qT[1], 1):
        th()
    emit_sc(1)
    for th in qk_unit_thunks(kT[1], 7):
        th()
    emit_v(range(0, NT))
    # q/k of same chunk adjacent: qT2,kT2,qT3,kT3,...
    units = [u for c in range(2, NK) for u in ((qT[c], c), (kT[c], c + 6))]
    for h in range(2, H):
        if h >= 4:  # chunk h//2 must be fully emitted before its scores
            fill.ensure(2 * (h // 2 - 2) + 1)
        emit_sc(h)
        if h - 2 < len(units):
            fill.add(qk_unit_thunks(*units[h - 2]), uid=h - 2)
        if h == 10:
            fill.add(tpdma_thunks([0, 1, 2, 3]))
        if h == 11:
            fill.add(tpdma_thunks([4]))
        emit_avT(h - 2, fill)
    # tail: avT(10) covered by proj partials; avT(11) finishes per tile
    fill.add(proj_partial_thunks(0))
    fill.add(proj_partial_thunks(1))
    emit_avT(10, fill)
    fill.drain()

    def tail_cb(t):
        tp_pe(5, t, tag="av")
        if t >= 1:
            proj_finish(t - 1)
            if t + 1 < NT:  # partials 0,1 are pre-queued before avT(10)
                fill.add(proj_partial_thunks(t + 1))
                fill.drain()

    emit_avT(11, fill, cb=tail_cb)
    proj_finish(NT - 1)


# pool buffer counts, patched onto tile_pool via tags at tile() time
_SB_BUFS = {
    "ident": 1, "ident32": 1, "tri": 1, "tri32": 1, "ones": 1, "ones32": 1,
    "bqk": 12, "bv32": 1, "bvrow": 1, "bp32": 1, "bprow": 1,
    "xT": 6, "qkT": 12, "v": 8, "ynat": 8, "yT": 6,
    "wa": 12, "wv": 1, "wp": 1,
    "expt": 3, "rc": 4, "osb": 4,
}
_PS_BUFS = {"qkv": 2, "sc": 2, "av": 2}


_NC_CACHE = {}


def _get_nc(has_battn, has_bproj, num_devices=N_CORES):
    key = (has_battn, has_bproj, num_devices)
    if key not in _NC_CACHE:
        _NC_CACHE[key] = _build_nc(has_battn, has_bproj, num_devices)
    return _NC_CACHE[key]


class _CachedSpmdRunner:
    """Jit the 8-core shard_map once; re-execute on every kernel() call.

    Mirrors concourse.bass2jax.run_bass_via_pjrt's multi-core path, minus the
    per-call retrace/recompile.
    """

    def __init__(self, nc, n_cores):
        import jax
        from jax.sharding import Mesh, PartitionSpec
        from jax.experimental.shard_map import shard_map
        from concourse import bass2jax

        bass2jax.install_neuronx_cc_hook()
        assert nc.dbg_addr is None
        self.n_cores = n_cores
        partition_name = (
            nc.partition_id_tensor.name if nc.partition_id_tensor else None
        )
        in_names, out_names, out_avals = [], [], []
        for alloc in nc.m.functions[0].allocations:
            if not isinstance(alloc, mybir.MemoryLocationSet):
                continue
            name = alloc.memorylocations[0].name
            if alloc.kind == "ExternalInput":
                if name != partition_name:
                    in_names.append(name)
            elif alloc.kind == "ExternalOutput":
                out_names.append(name)
                out_avals.append(
                    jax.core.ShapedArray(
                        tuple(alloc.tensor_shape), mybir.dt.np(alloc.dtype)
                    )
                )
        self.in_names, self.out_names, self.out_avals = (
            in_names, out_names, out_avals,
        )
        all_in_names = list(in_names) + out_names
        if partition_name is not None:
            all_in_names.append(partition_name)

        def _body(*args):
            operands = list(args)
            if partition_name is not None:
                operands.append(bass2jax.partition_id_tensor())
            return tuple(
                bass2jax._bass_exec_p.bind(
                    *operands,
                    out_avals=tuple(out_avals),
                    in_names=tuple(all_in_names),
                    out_names=tuple(out_names),
                    lowering_input_output_aliases=(),
                    sim_require_finite=True,
                    sim_require_nnan=True,
                    nc=nc,
                )
            )

        import numpy as _np

        devices = jax.devices()[:n_cores]
        mesh = Mesh(_np.asarray(devices), ("core",))
        n_io = len(in_names) + len(out_names)
        self.fn = jax.jit(
            shard_map(
                _body,
                mesh=mesh,
                in_specs=(PartitionSpec("core"),) * n_io,
                out_specs=(PartitionSpec("core"),) * len(out_names),
                check_rep=False,
            ),
            keep_unused=True,
        )
        self.sharding = jax.sharding.NamedSharding(mesh, PartitionSpec("core"))

    def run(self, in_maps):
        import jax

        args = []
        for name in self.in_names:
            cat = np.concatenate(
                [np.asarray(m[name]) for m in in_maps], axis=0
            )
            args.append(jax.device_put(cat, self.sharding))
        for av in self.out_avals:
            z = np.zeros((self.n_cores * av.shape[0], *av.shape[1:]), av.dtype)
            args.append(jax.device_put(z, self.sharding))
        outs = self.fn(*args)
        return [
            {
                name: np.asarray(outs[i]).reshape(
                    self.n_cores, *self.out_avals[i].shape
                )[c]
                for i, name in enumerate(self.out_names)
            }
            for c in range(self.n_cores)
        ]


_RUNNER_CACHE = {}


def _bf16(a):
    import ml_dtypes

    return np.ascontiguousarray(np.asarray(a).astype(ml_dtypes.bfloat16))


def kernel(x, W_attn, b_attn, W_proj, b_proj):
    x = _bf16(np.asarray(x).transpose(0, 2, 1))  # [B, C, T]: device wants x^T
    W_attn = _bf16(W_attn)
    W_proj = _bf16(W_proj)
    b_attn = np.asarray(b_attn, dtype=np.float32)
    b_proj = np.asarray(b_proj, dtype=np.float32)
    has_battn = bool(np.any(b_attn != 0.0))
    has_bproj = bool(np.any(b_proj != 0.0))

    nc = _get_nc(has_battn, has_bproj)
    in_maps = []
    for b in range(N_CORES):
        m = {"x": x[b], "W_attn": W_attn, "W_proj": W_proj}
        if has_battn:
            m["b_attn"] = b_attn
        if has_bproj:
            m["b_proj"] = b_proj
        in_maps.append(m)

    key = (has_battn, has_bproj)
    try:
        if key not in _RUNNER_CACHE:
            _RUNNER_CACHE[key] = _CachedSpmdRunner(nc, N_CORES)
        results = _RUNNER_CACHE[key].run(in_maps)
    except Exception:
        results = run_bass_kernel_spmd(
            nc, in_maps, core_ids=list(range(N_CORES))
        ).results
    return np.stack([results[b]["y"] for b in range(N_CORES)]).astype(np.float32)


# revision 65
# speedup vs baseline: 1.3421x; 1.0171x over previous
"""Trainium2 Bass kernel for causal multi-head attention block.

Module: qkv = x @ W_attn + b_attn; causal softmax((q k^T)/sqrt(C)); y = (attn @ v) @ W_proj + b_proj
Shapes (hardcoded): x [8, 1024, 768], W_attn [768, 2304], W_proj [768, 768], H=12, D=64.

Sharding: data parallel over batch - core b computes batch element b (B == n_cores == 8).
No collectives; host scatters x and gathers y. Host pre-casts x/W to bf16.

Per-core dataflow (all matmul operands bf16, fp32 PSUM accumulate):
  1. x arrives host-pre-transposed as x^T [768,1024] bf16; 6 plain loads
  2. QT,KT [c,1024] = (W_attn chunk).T @ xT   (W stationary, xT moving)
     V natural [tok, 12*(64+1)]: per head [V_h | 1.0] so the AV matmul emits
     the softmax denominator as the 65th output column.
  3. per head h, blocks (query-group g, key-tile j) packed first-fit-decreasing
     into full 512-word PSUM bins -> 5 exp instructions/head:
       scoresT[tk,tq] = KT_h.T @ QT_h   (causal-tight tiles only)
       e = Exp(scoresT/sqrt(768))       (ScalarE, PSUM->SBUF bf16)
       diag tiles *= upper-tri mask     (Pool)
       avT[tq, 65] += e_block.T @ [V_h|1]  (65 moving rows/tile-pair: ~2x
                                            fewer PE rows than 512-wide AV)
       Y[t, h*64:] = avT[:,0:64] * recip(avT[:,64])  (normalization fused
                                            into the PSUM->SBUF copy, DVE)
     Scores run two heads ahead of AV; QK-chunk matmuls interleave as PE
     filler so exp latency and fin pacing never stall the PE.
  4. Y -> yT via PE transposes; out[tq, c] = yT.T @ W_proj (+bias), DMA out.
"""

import math
from collections import deque

import numpy as np

import concourse.bass as bass
import concourse.mybir as mybir
from concourse import bacc, tile
from concourse.bass_utils import run_bass_kernel_spmd
from concourse.masks import make_identity, make_upper_triangular

N_CORES = 8
B, T, C = 8, 1024, 768
H, D = 12, 64
C3 = 3 * C
NT = T // 128          # 8 token tiles
NK = C // 128          # 6 contraction chunks
NG = T // 512          # 2 query groups of 512
SCALE = 1.0 / math.sqrt(C)

F32 = mybir.dt.float32
BF16 = mybir.dt.bfloat16
MULT = mybir.AluOpType.mult
EXP = mybir.ActivationFunctionType.Exp


def _head_plan():
    """Causal score blocks for one head, packed into PSUM tiles.

    Returns (tiles, ecol): tiles is a list of (blocks, tile_width) where each
    block is (g, j, nw, tq0, diag, sc_off, e_off); ecol[(t, j)] gives the
    column in the head's e tile holding scoresT[:, tq-tile t] for key-tile j.
    """
    blocks = []
    for g in range(NG):
        for j in range(4 * g + 4):
            nw = 512 - max(j * 128 - g * 512, 0)
            tq0 = g * 512 + (512 - nw)
            blocks.append([g, j, nw, tq0])
    # first-fit-decreasing into 512-wide bins (bins end up exactly full)
    blocks.sort(key=lambda b: -b[2])
    bins = []
    for b in blocks:
        for bn in bins:
            if sum(x[2] for x in bn) + b[2] <= 512:
                bn.append(b)
                break
        else:
            bins.append([b])
    # pair bins into [128,1024] PSUM tiles; assign sc/e offsets
    tiles = []
    e_base = 0
    for i in range(0, len(bins), 2):
        pair = bins[i : i + 2]
        tb, off = [], 0
        for bn in pair:
            for g, j, nw, tq0 in bn:
                diag = j * 128 == tq0
                tb.append((g, j, nw, tq0, diag, off, e_base + off))
                off += nw
        tiles.append((tb, off))
        e_base += off
    assert e_base == 4608
    ecol = {}
    for tb, _ in tiles:
        for g, j, nw, tq0, diag, sc_off, e_off in tb:
            for t in range(4 * g, 4 * g + 4):
                lo = t * 128
                if tq0 <= lo and lo + 128 <= tq0 + nw:
                    ecol[(t, j)] = e_off + lo - tq0
    for t in range(NT):
        for j in range(t + 1):
            assert (t, j) in ecol, (t, j)
    return tiles, ecol


_PLAN_TILES, _PLAN_ECOL = _head_plan()


def _build_nc(has_battn: bool, has_bproj: bool, num_devices: int, repeat: int = 1):
    nc = bacc.Bacc(
        "TRN2", target_bir_lowering=False, debug=False, num_devices=num_devices
    )

    x_d = nc.dram_tensor("x", (C, T), BF16, kind="ExternalInput")  # pre-transposed on host
    wa_d = nc.dram_tensor("W_attn", (C, C3), BF16, kind="ExternalInput")
    wp_d = nc.dram_tensor("W_proj", (C, C), BF16, kind="ExternalInput")
    ba_d = bp_d = None
    if has_battn:
        ba_d = nc.dram_tensor("b_attn", (C3,), F32, kind="ExternalInput")
    if has_bproj:
        bp_d = nc.dram_tensor("b_proj", (C,), F32, kind="ExternalInput")
    y_d = nc.dram_tensor("y", (T, C), F32, kind="ExternalOutput")

    with tile.TileContext(nc) as tc:
        with (
            tc.tile_pool(name="sb", bufs=1) as sb,
            tc.tile_pool(name="ps", bufs=1, space="PSUM") as ps,
        ):
            for r in range(repeat):
                _emit(nc, sb, ps, x_d, wa_d, wp_d, ba_d, bp_d, y_d,
                      warmup=(r == 0))
    nc.compile()
    return nc


def _emit(nc, sb, ps, x_d, wa_d, wp_d, ba_d, bp_d, y_d, warmup=True):
    def st(shape, tag, dtype=BF16, name=None):
        return sb.tile(list(shape), dtype, tag=tag, bufs=_SB_BUFS[tag],
                       name=name or tag)

    def pt(shape, tag, dtype=F32, name=None):
        return ps.tile(list(shape), dtype, tag=tag, bufs=_PS_BUFS[tag],
                       name=name or tag)

    # --- constants ---
    ident32 = st([128, 128], "ident32", dtype=F32)
    make_identity(nc, ident32[:])
    ident = st([128, 128], "ident")
    nc.vector.tensor_copy(ident[:], ident32[:])
    tri32 = st([128, 128], "tri32", dtype=F32)
    make_upper_triangular(nc, tri32[:], val=1.0, diag=True)  # tri[r,c]=1 iff c>=r
    tri = st([128, 128], "tri")
    nc.vector.tensor_copy(tri[:], tri32[:])

    bqk = []
    ones = bv_row = bp_row = None
    if ba_d is not None or bp_d is not None:
        ones32 = st([1, 128], "ones32", dtype=F32)
        nc.gpsimd.memset(ones32[:], 1.0)
        ones = st([1, 128], "ones")
        nc.vector.tensor_copy(ones[:], ones32[:])
    if ba_d is not None:
        for m in range(12):  # Q,K outchan tiles 0..1535
            bt = st([128, 1], "bqk", dtype=F32)
            nc.sync.dma_start(bt[:], ba_d.ap()[m * 128 : (m + 1) * 128])
            bqk.append(bt)
        bv32 = st([1, C], "bv32", dtype=F32)
        nc.sync.dma_start(bv32[:], ba_d.ap()[2 * C : 3 * C])
        bv_row = st([1, C], "bvrow")
        nc.vector.tensor_copy(bv_row[:], bv32[:])
    if bp_d is not None:
        bp32 = st([1, C], "bp32", dtype=F32)
        nc.sync.dma_start(bp32[:], bp_d.ap())
        bp_row = st([1, C], "bprow")
        nc.vector.tensor_copy(bp_row[:], bp32[:])

    # --- persistent activations ---
    qT = [st([128, T], "qkT", name=f"qT{c}") for c in range(NK)]
    kT = [st([128, T], "qkT", name=f"kT{c}") for c in range(NK)]
    vt = [st([128, H * 65], "v", name=f"v{t}") for t in range(NT)]
    Yn = [st([128, C], "ynat", name=f"Y{t}") for t in range(NT)]
    yTc = [st([128, T], "yT", name=f"yT{c}") for c in range(NK)]

    # --- DMAs (SP queue; emission order = issue order) ---
    was = []

    def load_wa(m):
        wam = st([128, NK * 128], "wa", name=f"wa{m}")
        nc.sync.dma_start(
            wam[:],
            wa_d.ap()[:, m * 128 : (m + 1) * 128].rearrange(
                "(k p) n -> p k n", p=128
            ),
        )
        return wam

    was.append(load_wa(0))      # Q chunk 0
    was.append(load_wa(6))      # K chunk 0
    xT = []
    for c in range(NK):
        xt = st([128, T], "xT", name=f"xT{c}")
        xT.append(xt)
        nc.sync.dma_start(xt[:], x_d.ap()[c * 128 : (c + 1) * 128, :])
    was.append(load_wa(1))
    was.append(load_wa(7))
    # all W_attn V columns, every k-chunk, in one 3-D-AP DMA
    wvall = st([128, NK * C], "wv")
    nc.sync.dma_start(
        wvall[:],
        wa_d.ap()[:, 2 * C : 3 * C].rearrange("(k p) n -> p k n", p=128),
    )
    for c in range(2, NK):
        was.append(load_wa(c))
        was.append(load_wa(c + 6))
    wa_of = {}
    order = [0, 6, 1, 7] + [m for c in range(2, NK) for m in (c, c + 6)]
    for i, m in enumerate(order):
        wa_of[m] = was[i]
    wpall = st([128, NK * C], "wp")
    nc.sync.dma_start(
        wpall[:], wp_d.ap().rearrange("(k p) n -> p k n", p=128)
    )

    # --- QK-chunk unit: thunk list (filler granularity = 1 matmul) ---
    def qk_unit_thunks(dst, m, cp_eng=None):
        ths = []
        for g in range(NG):
            box = {}

            def mk(k, g=g, box=box):
                def run():
                    if k == 0:
                        box["acc"] = pt([128, 512], "qkv")
                    nc.tensor.matmul(
                        box["acc"][:],
                        wa_of[m][:, k * 128 : (k + 1) * 128],
                        xT[k][:, g * 512 : (g + 1) * 512],
                        start=(k == 0),
                        stop=(k == NK - 1),
                    )
                return run

            ths += [mk(k) for k in range(NK)]

            def cp(g=g, box=box, m=m):
                dst_ap = dst[:, g * 512 : (g + 1) * 512]
                if ba_d is not None:
                    nc.vector.tensor_scalar_add(dst_ap, box["acc"][:], bqk[m][:])
                elif cp_eng == "act":
                    nc.scalar.copy(dst_ap, box["acc"][:])
                else:
                    nc.vector.tensor_copy(dst_ap, box["acc"][:])

            ths.append(cp)
        return ths

    # --- V natural [tok, 12*(64+1)] ---
    def emit_v(trange):
        for t in trange:
            nc.gpsimd.memset(vt[t][:], 1.0)  # ones cols; data cols overwritten
            for co, w in ((0, 512), (512, 256)):
                acc = pt([128, w], "qkv")
                for k in range(NK):
                    nc.tensor.matmul(
                        acc[:],
                        xT[k][:, t * 128 : (t + 1) * 128],
                        wvall[:, k * C + co : k * C + co + w],
                        start=(k == 0),
                        stop=(k == NK - 1) and ba_d is None,
                    )
                if ba_d is not None:
                    nc.tensor.matmul(
                        acc[:],
                        ones[0:1, 0:128],
                        bv_row[0:1, co : co + w],
                        start=False,
                        stop=True,
                    )
                nc.scalar.copy(
                    vt[t][:].rearrange("p (h e) -> p h e", e=65)[
                        :, co // D : (co + w) // D, 0:D
                    ],
                    acc[:].rearrange("p (h e) -> p h e", e=D),
                )

    # --- per-head scores + exp + mask ---
    e_of = {}

    def emit_sc(h):
        ch, off = h // 2, (h % 2) * 64
        e = st([128, 4608], "expt", name=f"e{h}")
        e_of[h] = e
        for tb, tw in _PLAN_TILES:
            sc = pt([128, 1024], "sc")
            for g, j, nw, tq0, diag, sc_off, e_off in tb:
                nc.tensor.matmul(
                    sc[:, sc_off : sc_off + nw],
                    kT[ch][off : off + 64, j * 128 : (j + 1) * 128],
                    qT[ch][off : off + 64, tq0 : tq0 + nw],
                    start=True,
                    stop=True,
                )
            e_base = tb[0][6]
            nc.scalar.activation(
                e[:, e_base : e_base + tw], sc[:, 0:tw], EXP, scale=SCALE
            )
            for g, j, nw, tq0, diag, sc_off, e_off in tb:
                if diag:
                    nc.gpsimd.tensor_tensor(
                        e[:, e_off : e_off + 128],
                        e[:, e_off : e_off + 128],
                        tri[:],
                        op=MULT,
                    )

    # --- per-head AV^T + fused-normalization fin, interleaved with filler ---
    def tp_pe(chh, t, tag="qkv"):
        ptt = pt([128, 128], tag, dtype=BF16)
        nc.tensor.transpose(
            ptt[:], Yn[t][:, chh * 128 : (chh + 1) * 128], ident[:]
        )
        nc.vector.tensor_copy(yTc[chh][:, t * 128 : (t + 1) * 128], ptt[:])

    def _avT_tile(h, t, e):
        av = pt([128, 65], "av")
        for j in range(t + 1):
            nc.tensor.matmul(
                av[:],
                e[:, _PLAN_ECOL[(t, j)] : _PLAN_ECOL[(t, j)] + 128],
                vt[j][:, h * 65 : (h + 1) * 65],
                start=(j == 0),
                stop=(j == t),
            )
        rc = st([128, 1], "rc", dtype=F32)
        nc.vector.reciprocal(rc[:], av[:, 64:65])
        nc.vector.tensor_scalar_mul(
            Yn[t][:, h * D : (h + 1) * D], av[:, 0:D], rc[:]
        )

    def emit_avT(h, filler, inline_tp=None, cb=None):
        e = e_of.pop(h)
        for t in range(NT):
            _avT_tile(h, t, e)
            if inline_tp is not None:
                tp_pe(inline_tp, t)
            if cb is not None:
                cb(t)
            filler.step(2 if t < 5 else 1)

    def emit_avT_pair(ha, hb, filler, inline_tp=(None, None)):
        ea, eb = e_of.pop(ha), e_of.pop(hb)
        for t in range(NT):
            _avT_tile(ha, t, ea)
            if inline_tp[0] is not None:
                tp_pe(inline_tp[0], t)
            filler.step(2 if t < 5 else 1)
            _avT_tile(hb, t, eb)
            if inline_tp[1] is not None:
                tp_pe(inline_tp[1], t)
            filler.step(2 if t < 5 else 1)

    class Filler:
        def __init__(self):
            self.q = deque()

        def add(self, ths, uid=10**9):
            self.q.extend((uid, th) for th in ths)

        def step(self, n=1):
            for _ in range(n):
                if self.q:
                    self.q.popleft()[1]()

        def ensure(self, uid):
            while self.q and self.q[0][0] <= uid:
                self.q.popleft()[1]()

        def drain(self):
            while self.q:
                self.q.popleft()[1]()

    # --- Y -> yT transpose via DMA XBAR (zero-PE thunks for early chunks) ---
    def tpdma_thunks(chs):
        ths = []
        for chh in chs:
            for t in range(NT):
                def run(chh=chh, t=t):
                    nc.sync.dma_start_transpose(
                        yTc[chh][:, t * 128 : (t + 1) * 128],
                        Yn[t][:, chh * 128 : (chh + 1) * 128],
                    )
                ths.append(run)
        return ths

    # --- output projection, split so proj fills the attention tail:
    # proj_partial(t) = co0 contraction k0..4 (needs only chunks 0..4);
    # proj_finish(t) = co0 k5 + full co1 + copies + stores (needs chunk 5,
    # i.e. fin(h10/h11, t) + the inline ch5 transpose).
    pacc = {}

    def proj_partial_thunks(t):
        ths = []

        def mk(k, t=t):
            def run():
                if k == 0:
                    pacc[t] = pt([128, 512], "qkv")
                nc.tensor.matmul(
                    pacc[t][:],
                    yTc[k][:, t * 128 : (t + 1) * 128],
                    wpall[:, k * C : k * C + 512],
                    start=(k == 0),
                    stop=False,
                )
            return run

        return [mk(k) for k in range(5)]

    def proj_finish(t):
        osb = st([128, C], "osb", dtype=F32)
        acc = pacc.pop(t)
        nc.tensor.matmul(
            acc[:],
            yTc[5][:, t * 128 : (t + 1) * 128],
            wpall[:, 5 * C : 5 * C + 512],
            start=False,
            stop=bp_d is None,
        )
        if bp_d is not None:
            nc.tensor.matmul(
                acc[:], ones[0:1, 0:128], bp_row[0:1, 0:512],
                start=False, stop=True,
            )
        nc.scalar.copy(osb[:, 0:512], acc[:])
        # store the big half as soon as its copy lands: only the small co1
        # strip transfers after the last compute
        nc.sync.dma_start(y_d.ap()[t * 128 : (t + 1) * 128, 0:512], osb[:, 0:512])
        acc1 = pt([128, 256], "sc")  # sc banks are free once exps are done
        for k in range(NK):
            nc.tensor.matmul(
                acc1[:],
                yTc[k][:, t * 128 : (t + 1) * 128],
                wpall[:, k * C + 512 : (k + 1) * C],
                start=(k == 0),
                stop=(k == NK - 1) and bp_d is None,
            )
        if bp_d is not None:
            nc.tensor.matmul(
                acc1[:], ones[0:1, 0:128], bp_row[0:1, 512:768],
                start=False, stop=True,
            )
        if t == NT - 1:
            nc.vector.tensor_copy(osb[:, 512:768], acc1[:])
        else:
            nc.scalar.copy(osb[:, 512:768], acc1[:])
        nc.sync.dma_start(
            y_d.ap()[t * 128 : (t + 1) * 128, 512:768], osb[:, 512:768]
        )

    # --- schedule ---
    # p-state warmup: the PE needs ~3us of continuous busy to reach max clock;
    # burn identity matmuls into the (otherwise idle) sc bank while the first
    # DMAs are in flight so real matmuls start at peak frequency.
    for i in range(37 if warmup else 0):
        wsc = pt([128, 1024], "sc") if i % 8 == 0 else wsc  # noqa: F821
        wid = ident32 if i < 3 else ident  # f32 rows are 4x slower: good early
        nc.tensor.matmul(
            wsc[:, (i % 8) * 128 : (i % 8 + 1) * 128],
            wid[:],
            wid[:],
            start=True,
            stop=True,
        )

    fill = Filler()
    # chunk 0 k-major: all four acc groups (qT0/kT0 x g0/g1) stream each x
    # chunk as it arrives; kT0 accs borrow the (still idle) av banks
    c0_accs = {}
    for k in range(NK):
        for dst, m, gg, tag in (
            (qT[0], 0, 0, "qkv"), (qT[0], 0, 1, "qkv"),
            (kT[0], 6, 0, "av"), (kT[0], 6, 1, "av"),
        ):
            key = (m, gg)
            if k == 0:
                c0_accs[key] = pt([128, 512], tag)
            nc.tensor.matmul(
                c0_accs[key][:],
                wa_of[m][:, k * 128 : (k + 1) * 128],
                xT[k][:, gg * 512 : (gg + 1) * 512],
                start=(k == 0),
                stop=(k == NK - 1),
            )
    for dst, m, gg in (
        (qT[0], 0, 0), (qT[0], 0, 1), (kT[0], 6, 0), (kT[0], 6, 1)
    ):
        dst_ap = dst[:, gg * 512 : (gg + 1) * 512]
        if ba_d is not None:
            nc.vector.tensor_scalar_add(dst_ap, c0_accs[(m, gg)][:], bqk[m][:])
        else:
            nc.scalar.copy(dst_ap, c0_accs[(m, gg)][:])
    emit_sc(0)
    for th in qk_unit_thunks(qT[1], 1):
        th()
    emit_sc(1)
    for th in qk_unit_thunks(kT[1], 7):
        th()
    emit_v(range(0, NT))
    # q/k of same chunk adjacent: qT2,kT2,qT3,kT3,...
    units = [u for c in range(2, NK) for u in ((qT[c], c), (kT[c], c + 6))]
    for h in range(2, H):
        if h >= 4:  # chunk h//2 must be fully emitted before its scores
            fill.ensure(2 * (h // 2 - 2) + 1)
        emit_sc(h)
        if h - 2 < len(units):
            fill.add(qk_unit_thunks(*units[h - 2]), uid=h - 2)
        if h == 10:
            fill.add(tpdma_thunks([0, 1, 2, 3]))
        if h == 11:
            fill.add(tpdma_thunks([4]))
        emit_avT(h - 2, fill)
    # tail: avT(10) covered by proj partials; avT(11) finishes per tile
    fill.add(proj_partial_thunks(0))
    fill.add(proj_partial_thunks(1))
    emit_avT(10, fill)
    fill.drain()

    def tail_cb(t):
        tp_pe(5, t, tag="av")
        if t >= 1:
            proj_finish(t - 1)
            if t + 1 < NT:  # partials 0,1 are pre-queued before avT(10)
                fill.add(proj_partial_thunks(t + 1))
                fill.drain()

    emit_avT(11, fill, cb=tail_cb)
    proj_finish(NT - 1)


# pool buffer counts, patched onto tile_pool via tags at tile() time
_SB_BUFS = {
    "ident": 1, "ident32": 1, "tri": 1, "tri32": 1, "ones": 1, "ones32": 1,
    "bqk": 12, "bv32": 1, "bvrow": 1, "bp32": 1, "bprow": 1,
    "xT": 6, "qkT": 12, "v": 8, "ynat": 8, "yT": 6,
    "wa": 12, "wv": 1, "wp": 1,
    "expt": 3, "rc": 4, "osb": 4,
}
_PS_BUFS = {"qkv": 2, "sc": 2, "av": 2}


_NC_CACHE = {}


def _get_nc(has_battn, has_bproj, num_devices=N_CORES):
    key = (has_battn, has_bproj, num_devices)
    if key not in _NC_CACHE:
        _NC_CACHE[key] = _build_nc(has_battn, has_bproj, num_devices)
    return _NC_CACHE[key]


class _CachedSpmdRunner:
    """Jit the 8-core shard_map once; re-execute on every kernel() call.

    Mirrors concourse.bass2jax.run_bass_via_pjrt's multi-core path, minus the
    per-call retrace/recompile.
    """

    def __init__(self, nc, n_cores):
        import jax
        from jax.sharding import Mesh, PartitionSpec
        from jax.experimental.shard_map import shard_map
        from concourse import bass2jax

        bass2jax.install_neuronx_cc_hook()
        assert nc.dbg_addr is None
        self.n_cores = n_cores
        partition_name = (
            nc.partition_id_tensor.name if nc.partition_id_tensor else None
        )
        in_names, out_names, out_avals = [], [], []
        for alloc in nc.m.functions[0].allocations:
            if not isinstance(alloc, mybir.MemoryLocationSet):
                continue
            name = alloc.memorylocations[0].name
            if alloc.kind == "ExternalInput":
                if name != partition_name:
                    in_names.append(name)
            elif alloc.kind == "ExternalOutput":
                out_names.append(name)
                out_avals.append(
                    jax.core.ShapedArray(
                        tuple(alloc.tensor_shape), mybir.dt.np(alloc.dtype)
                    )
                )
        self.in_names, self.out_names, self.out_avals = (
            in_names, out_names, out_avals,
        )
        all_in_names = list(in_names) + out_names
        if partition_name is not None:
            all_in_names.append(partition_name)

        def _body(*args):
            operands = list(args)
            if partition_name is not None:
                operands.append(bass2jax.partition_id_tensor())
            return tuple(
                bass2jax._bass_exec_p.bind(
                    *operands,
                    out_avals=tuple(out_avals),
                    in_names=tuple(all_in_names),
                    out_names=tuple(out_names),
                    lowering_input_output_aliases=(),
                    sim_require_finite=True,
                    sim_require_nnan=True,
                    nc=nc,
                )
            )

        import numpy as _np

        devices = jax.devices()[:n_cores]
        mesh = Mesh(_np.asarray(devices), ("core",))
        n_io = len(in_names) + len(out_names)
        self.fn = jax.jit(
            shard_map(
                _body,
                mesh=mesh,
                in_specs=(PartitionSpec("core"),) * n_io,
                out_specs=(PartitionSpec("core"),) * len(out_names),
                check_rep=False,
            ),
            keep_unused=True,
        )
        self.sharding = jax.sharding.NamedSharding(mesh, PartitionSpec("core"))

    def run(self, in_maps):
        import jax

        args = []
        for name in self.in_names:
            cat = np.concatenate(
                [np.asarray(m[name]) for m in in_maps], axis=0
            )
            args.append(jax.device_put(cat, self.sharding))
        for av in self.out_avals:
            z = np.zeros((self.n_cores * av.shape[0], *av.shape[1:]), av.dtype)
            args.append(jax.device_put(z, self.sharding))
        outs = self.fn(*args)
        return [
            {
                name: np.asarray(outs[i]).reshape(
                    self.n_cores, *self.out_avals[i].shape
                )[c]
                for i, name in enumerate(self.out_names)
            }
            for c in range(self.n_cores)
        ]


_RUNNER_CACHE = {}


def _bf16(a):
    import ml_dtypes

    return np.ascontiguousarray(np.asarray(a).astype(ml_dtypes.bfloat16))


def kernel(x, W_attn, b_attn, W_proj, b_proj):
    x = _bf16(np.asarray(x).transpose(0, 2, 1))  # [B, C, T]: device wants x^T
    W_attn = _bf16(W_attn)
    W_proj = _bf16(W_proj)
    b_attn = np.asarray(b_attn, dtype=np.float32)
    b_proj = np.asarray(b_proj, dtype=np.float32)
    has_battn = bool(np.any(b_attn != 0.0))
    has_bproj = bool(np.any(b_proj != 0.0))

    nc = _get_nc(has_battn, has_bproj)
    in_maps = []
    for b in range(N_CORES):
        m = {"x": x[b], "W_attn": W_attn, "W_proj": W_proj}
        if has_battn:
            m["b_attn"] = b_attn
        if has_bproj:
            m["b_proj"] = b_proj
        in_maps.append(m)

    key = (has_battn, has_bproj)
    try:
        if key not in _RUNNER_CACHE:
            _RUNNER_CACHE[key] = _CachedSpmdRunner(nc, N_CORES)
        results = _RUNNER_CACHE[key].run(in_maps)
    except Exception:
        results = run_bass_kernel_spmd(
            nc, in_maps, core_ids=list(range(N_CORES))
        ).results
    return np.stack([results[b]["y"] for b in range(N_CORES)]).astype(np.float32)


# revision 74
# speedup vs baseline: 1.3463x; 1.0031x over previous
"""Trainium2 Bass kernel for causal multi-head attention block.

Module: qkv = x @ W_attn + b_attn; causal softmax((q k^T)/sqrt(C)); y = (attn @ v) @ W_proj + b_proj
Shapes (hardcoded): x [8, 1024, 768], W_attn [768, 2304], W_proj [768, 768], H=12, D=64.

Sharding: data parallel over batch - core b computes batch element b (B == n_cores == 8).
No collectives; host scatters x and gathers y. Host pre-casts x/W to bf16.

Per-core dataflow (all matmul operands bf16, fp32 PSUM accumulate):
  1. x arrives host-pre-transposed as x^T [768,1024] bf16; 6 plain loads
  2. QT,KT [c,1024] = (W_attn chunk).T @ xT   (W stationary, xT moving)
     V natural [tok, 12*(64+1)]: per head [V_h | 1.0] so the AV matmul emits
     the softmax denominator as the 65th output column.
  3. per head h, blocks (query-group g, key-tile j) packed first-fit-decreasing
     into full 512-word PSUM bins -> 5 exp instructions/head:
       scoresT[tk,tq] = KT_h.T @ QT_h   (causal-tight tiles only)
       e = Exp(scoresT/sqrt(768))       (ScalarE, PSUM->SBUF bf16)
       diag tiles *= upper-tri mask     (Pool)
       avT[tq, 65] += e_block.T @ [V_h|1]  (65 moving rows/tile-pair: ~2x
                                            fewer PE rows than 512-wide AV)
       Y[t, h*64:] = avT[:,0:64] * recip(avT[:,64])  (normalization fused
                                            into the PSUM->SBUF copy, DVE)
     Scores run two heads ahead of AV; QK-chunk matmuls interleave as PE
     filler so exp latency and fin pacing never stall the PE.
  4. Y -> yT via PE transposes; out[tq, c] = yT.T @ W_proj (+bias), DMA out.
"""

import math
from collections import deque

import numpy as np

import concourse.bass as bass
import concourse.mybir as mybir
from concourse import bacc, tile
from concourse.bass_utils import run_bass_kernel_spmd
from concourse.masks import make_identity, make_upper_triangular

N_CORES = 8
B, T, C = 8, 1024, 768
H, D = 12, 64
C3 = 3 * C
NT = T // 128          # 8 token tiles
NK = C // 128          # 6 contraction chunks
NG = T // 512          # 2 query groups of 512
SCALE = 1.0 / math.sqrt(C)

F32 = mybir.dt.float32
BF16 = mybir.dt.bfloat16
MULT = mybir.AluOpType.mult
EXP = mybir.ActivationFunctionType.Exp


def _head_plan():
    """Causal score blocks for one head, packed into PSUM tiles.

    Returns (tiles, ecol): tiles is a list of (blocks, tile_width) where each
    block is (g, j, nw, tq0, diag, sc_off, e_off); ecol[(t, j)] gives the
    column in the head's e tile holding scoresT[:, tq-tile t] for key-tile j.
    """
    blocks = []
    for g in range(NG):
        for j in range(4 * g + 4):
            nw = 512 - max(j * 128 - g * 512, 0)
            tq0 = g * 512 + (512 - nw)
            blocks.append([g, j, nw, tq0])
    # first-fit-decreasing into 512-wide bins (bins end up exactly full)
    blocks.sort(key=lambda b: -b[2])
    bins = []
    for b in blocks:
        for bn in bins:
            if sum(x[2] for x in bn) + b[2] <= 512:
                bn.append(b)
                break
        else:
            bins.append([b])
    # pair bins into [128,1024] PSUM tiles; assign sc/e offsets
    tiles = []
    e_base = 0
    for i in range(0, len(bins), 2):
        pair = bins[i : i + 2]
        tb, off = [], 0
        for bn in pair:
            for g, j, nw, tq0 in bn:
                diag = j * 128 == tq0
                tb.append((g, j, nw, tq0, diag, off, e_base + off))
                off += nw
        tiles.append((tb, off))
        e_base += off
    assert e_base == 4608
    ecol = {}
    for tb, _ in tiles:
        for g, j, nw, tq0, diag, sc_off, e_off in tb:
            for t in range(4 * g, 4 * g + 4):
                lo = t * 128
                if tq0 <= lo and lo + 128 <= tq0 + nw:
                    ecol[(t, j)] = e_off + lo - tq0
    for t in range(NT):
        for j in range(t + 1):
            assert (t, j) in ecol, (t, j)
    return tiles, ecol


_PLAN_TILES, _PLAN_ECOL = _head_plan()


def _build_nc(has_battn: bool, has_bproj: bool, num_devices: int, repeat: int = 1):
    nc = bacc.Bacc(
        "TRN2", target_bir_lowering=False, debug=False, num_devices=num_devices
    )

    x_d = nc.dram_tensor("x", (C, T), BF16, kind="ExternalInput")  # pre-transposed on host
    wa_d = nc.dram_tensor("W_attn", (C, C3), BF16, kind="ExternalInput")
    wp_d = nc.dram_tensor("W_proj", (C, C), BF16, kind="ExternalInput")
    ba_d = bp_d = None
    if has_battn:
        ba_d = nc.dram_tensor("b_attn", (C3,), F32, kind="ExternalInput")
    if has_bproj:
        bp_d = nc.dram_tensor("b_proj", (C,), F32, kind="ExternalInput")
    y_d = nc.dram_tensor("y", (T, C), F32, kind="ExternalOutput")

    with tile.TileContext(nc) as tc:
        with (
            tc.tile_pool(name="sb", bufs=1) as sb,
            tc.tile_pool(name="ps", bufs=1, space="PSUM") as ps,
        ):
            for r in range(repeat):
                _emit(nc, sb, ps, x_d, wa_d, wp_d, ba_d, bp_d, y_d,
                      warmup=(r == 0))
    nc.compile()
    return nc


def _emit(nc, sb, ps, x_d, wa_d, wp_d, ba_d, bp_d, y_d, warmup=True):
    def st(shape, tag, dtype=BF16, name=None):
        return sb.tile(list(shape), dtype, tag=tag, bufs=_SB_BUFS[tag],
                       name=name or tag)

    def pt(shape, tag, dtype=F32, name=None):
        return ps.tile(list(shape), dtype, tag=tag, bufs=_PS_BUFS[tag],
                       name=name or tag)

    # --- constants ---
    ident32 = st([128, 128], "ident32", dtype=F32)
    make_identity(nc, ident32[:])
    ident = st([128, 128], "ident")
    nc.vector.tensor_copy(ident[:], ident32[:])
    tri32 = st([128, 128], "tri32", dtype=F32)
    make_upper_triangular(nc, tri32[:], val=1.0, diag=True)  # tri[r,c]=1 iff c>=r
    tri = st([128, 128], "tri")
    nc.vector.tensor_copy(tri[:], tri32[:])

    bqk = []
    ones = bv_row = bp_row = None
    if ba_d is not None or bp_d is not None:
        ones32 = st([1, 128], "ones32", dtype=F32)
        nc.gpsimd.memset(ones32[:], 1.0)
        ones = st([1, 128], "ones")
        nc.vector.tensor_copy(ones[:], ones32[:])
    if ba_d is not None:
        for m in range(12):  # Q,K outchan tiles 0..1535
            bt = st([128, 1], "bqk", dtype=F32)
            nc.sync.dma_start(bt[:], ba_d.ap()[m * 128 : (m + 1) * 128])
            bqk.append(bt)
        bv32 = st([1, C], "bv32", dtype=F32)
        nc.sync.dma_start(bv32[:], ba_d.ap()[2 * C : 3 * C])
        bv_row = st([1, C], "bvrow")
        nc.vector.tensor_copy(bv_row[:], bv32[:])
    if bp_d is not None:
        bp32 = st([1, C], "bp32", dtype=F32)
        nc.sync.dma_start(bp32[:], bp_d.ap())
        bp_row = st([1, C], "bprow")
        nc.vector.tensor_copy(bp_row[:], bp32[:])

    # --- persistent activations ---
    qT = [st([128, T], "qkT", name=f"qT{c}") for c in range(NK)]
    kT = [st([128, T], "qkT", name=f"kT{c}") for c in range(NK)]
    vt = [st([128, H * 65], "v", name=f"v{t}") for t in range(NT)]
    Yn = [st([128, C], "ynat", name=f"Y{t}") for t in range(NT)]
    yTc = [st([128, T], "yT", name=f"yT{c}") for c in range(NK)]

    # --- DMAs (SP queue; emission order = issue order) ---
    was = []

    def load_wa(m):
        wam = st([128, NK * 128], "wa", name=f"wa{m}")
        nc.sync.dma_start(
            wam[:],
            wa_d.ap()[:, m * 128 : (m + 1) * 128].rearrange(
                "(k p) n -> p k n", p=128
            ),
        )
        return wam

    was.append(load_wa(0))      # Q chunk 0
    was.append(load_wa(6))      # K chunk 0
    xT = []
    for c in range(NK):
        xt = st([128, T], "xT", name=f"xT{c}")
        xT.append(xt)
        nc.sync.dma_start(xt[:], x_d.ap()[c * 128 : (c + 1) * 128, :])
    was.append(load_wa(1))
    was.append(load_wa(7))
    # all W_attn V columns, every k-chunk, in one 3-D-AP DMA
    wvall = st([128, NK * C], "wv")
    nc.sync.dma_start(
        wvall[:],
        wa_d.ap()[:, 2 * C : 3 * C].rearrange("(k p) n -> p k n", p=128),
    )
    for c in range(2, NK):
        was.append(load_wa(c))
        was.append(load_wa(c + 6))
    wa_of = {}
    order = [0, 6, 1, 7] + [m for c in range(2, NK) for m in (c, c + 6)]
    for i, m in enumerate(order):
        wa_of[m] = was[i]
    wpall = st([128, NK * C], "wp")
    nc.sync.dma_start(
        wpall[:], wp_d.ap().rearrange("(k p) n -> p k n", p=128)
    )

    # --- QK-chunk unit: thunk list (filler granularity = 1 matmul) ---
    def qk_unit_thunks(dst, m, cp_eng=None):
        ths = []
        for g in range(NG):
            box = {}

            def mk(k, g=g, box=box):
                def run():
                    if k == 0:
                        box["acc"] = pt([128, 512], "qkv")
                    nc.tensor.matmul(
                        box["acc"][:],
                        wa_of[m][:, k * 128 : (k + 1) * 128],
                        xT[k][:, g * 512 : (g + 1) * 512],
                        start=(k == 0),
                        stop=(k == NK - 1),
                    )
                return run

            ths += [mk(k) for k in range(NK)]

            def cp(g=g, box=box, m=m):
                dst_ap = dst[:, g * 512 : (g + 1) * 512]
                if ba_d is not None:
                    nc.vector.tensor_scalar_add(dst_ap, box["acc"][:], bqk[m][:])
                elif cp_eng == "act":
                    nc.scalar.copy(dst_ap, box["acc"][:])
                else:
                    nc.vector.tensor_copy(dst_ap, box["acc"][:])

            ths.append(cp)
        return ths

    # --- V natural [tok, 12*(64+1)] ---
    def emit_v(trange):
        for t in trange:
            nc.gpsimd.memset(vt[t][:], 1.0)  # ones cols; data cols overwritten
            for co, w in ((0, 512), (512, 256)):
                acc = pt([128, w], "qkv")
                for k in range(NK):
                    nc.tensor.matmul(
                        acc[:],
                        xT[k][:, t * 128 : (t + 1) * 128],
                        wvall[:, k * C + co : k * C + co + w],
                        start=(k == 0),
                        stop=(k == NK - 1) and ba_d is None,
                    )
                if ba_d is not None:
                    nc.tensor.matmul(
                        acc[:],
                        ones[0:1, 0:128],
                        bv_row[0:1, co : co + w],
                        start=False,
                        stop=True,
                    )
                nc.scalar.copy(
                    vt[t][:].rearrange("p (h e) -> p h e", e=65)[
                        :, co // D : (co + w) // D, 0:D
                    ],
                    acc[:].rearrange("p (h e) -> p h e", e=D),
                )

    # --- per-head scores + exp + mask ---
    e_of = {}

    def emit_sc(h):
        ch, off = h // 2, (h % 2) * 64
        e = st([128, 4608], "expt", name=f"e{h}")
        e_of[h] = e
        for tb, tw in _PLAN_TILES:
            sc = pt([128, 1024], "sc")
            for g, j, nw, tq0, diag, sc_off, e_off in tb:
                nc.tensor.matmul(
                    sc[:, sc_off : sc_off + nw],
                    kT[ch][off : off + 64, j * 128 : (j + 1) * 128],
                    qT[ch][off : off + 64, tq0 : tq0 + nw],
                    start=True,
                    stop=True,
                )
            e_base = tb[0][6]
            nc.scalar.activation(
                e[:, e_base : e_base + tw], sc[:, 0:tw], EXP, scale=SCALE
            )
            for g, j, nw, tq0, diag, sc_off, e_off in tb:
                if diag:
                    nc.gpsimd.tensor_tensor(
                        e[:, e_off : e_off + 128],
                        e[:, e_off : e_off + 128],
                        tri[:],
                        op=MULT,
                    )

    # --- per-head AV^T + fused-normalization fin, interleaved with filler ---
    def tp_pe(chh, t, tag="qkv"):
        ptt = pt([128, 128], tag, dtype=BF16)
        nc.tensor.transpose(
            ptt[:], Yn[t][:, chh * 128 : (chh + 1) * 128], ident[:]
        )
        nc.vector.tensor_copy(yTc[chh][:, t * 128 : (t + 1) * 128], ptt[:])

    def _avT_tile(h, t, e):
        av = pt([128, 65], "av")
        for j in range(t + 1):
            nc.tensor.matmul(
                av[:],
                e[:, _PLAN_ECOL[(t, j)] : _PLAN_ECOL[(t, j)] + 128],
                vt[j][:, h * 65 : (h + 1) * 65],
                start=(j == 0),
                stop=(j == t),
            )
        rc = st([128, 1], "rc", dtype=F32)
        nc.vector.reciprocal(rc[:], av[:, 64:65])
        nc.vector.tensor_scalar_mul(
            Yn[t][:, h * D : (h + 1) * D], av[:, 0:D], rc[:]
        )

    def emit_avT(h, filler, inline_tp=None, cb=None):
        e = e_of.pop(h)
        for t in range(NT):
            _avT_tile(h, t, e)
            if inline_tp is not None:
                tp_pe(inline_tp, t)
            if cb is not None:
                cb(t)
            filler.step(2 if t < 5 else 1)

    def emit_avT_pair(ha, hb, filler, inline_tp=(None, None)):
        ea, eb = e_of.pop(ha), e_of.pop(hb)
        for t in range(NT):
            _avT_tile(ha, t, ea)
            if inline_tp[0] is not None:
                tp_pe(inline_tp[0], t)
            filler.step(2 if t < 5 else 1)
            _avT_tile(hb, t, eb)
            if inline_tp[1] is not None:
                tp_pe(inline_tp[1], t)
            filler.step(2 if t < 5 else 1)

    class Filler:
        def __init__(self):
            self.q = deque()

        def add(self, ths, uid=10**9):
            self.q.extend((uid, th) for th in ths)

        def step(self, n=1):
            for _ in range(n):
                if self.q:
                    self.q.popleft()[1]()

        def ensure(self, uid):
            while self.q and self.q[0][0] <= uid:
                self.q.popleft()[1]()

        def drain(self):
            while self.q:
                self.q.popleft()[1]()

    # --- Y -> yT transpose via DMA XBAR (zero-PE thunks for early chunks) ---
    def tpdma_thunks(chs):
        ths = []
        for chh in chs:
            for t in range(NT):
                def run(chh=chh, t=t):
                    nc.sync.dma_start_transpose(
                        yTc[chh][:, t * 128 : (t + 1) * 128],
                        Yn[t][:, chh * 128 : (chh + 1) * 128],
                    )
                ths.append(run)
        return ths

    # --- output projection, split so proj fills the attention tail:
    # proj_partial(t) = co0 contraction k0..4 (needs only chunks 0..4);
    # proj_finish(t) = co0 k5 + full co1 + copies + stores (needs chunk 5,
    # i.e. fin(h10/h11, t) + the inline ch5 transpose).
    pacc = {}

    def proj_partial_thunks(t, ks=range(5)):
        ths = []

        def mk(k, t=t):
            def run():
                if k == 0:
                    pacc[t] = pt([128, 512], "qkv")
                nc.tensor.matmul(
                    pacc[t][:],
                    yTc[k][:, t * 128 : (t + 1) * 128],
                    wpall[:, k * C : k * C + 512],
                    start=(k == 0),
                    stop=False,
                )
            return run

        return [mk(k) for k in ks]

    pacc1 = {}

    def proj_partial1_thunks(t, ks=range(3)):
        def mk(k, t=t):
            def run():
                if k == 0:
                    pacc1[t] = pt([128, 256], "sc")
                nc.tensor.matmul(
                    pacc1[t][:],
                    yTc[k][:, t * 128 : (t + 1) * 128],
                    wpall[:, k * C + 512 : (k + 1) * C],
                    start=(k == 0),
                    stop=False,
                )
            return run

        return [mk(k) for k in ks]

    def proj_finish(t):
        osb = st([128, C], "osb", dtype=F32)
        acc = pacc.pop(t)
        for k in (3, 4, 5):  # k3/k4 here so partials never wait late yT chunks
            nc.tensor.matmul(
                acc[:],
                yTc[k][:, t * 128 : (t + 1) * 128],
                wpall[:, k * C : k * C + 512],
                start=False,
                stop=(k == 5) and bp_d is None,
            )
        if bp_d is not None:
            nc.tensor.matmul(
                acc[:], ones[0:1, 0:128], bp_row[0:1, 0:512],
                start=False, stop=True,
            )
        nc.scalar.copy(osb[:, 0:512], acc[:])
        # store the big half as soon as its copy lands: only the small co1
        # strip transfers after the last compute
        nc.sync.dma_start(y_d.ap()[t * 128 : (t + 1) * 128, 0:512], osb[:, 0:512])
        acc1 = pt([128, 256], "sc")  # sc banks are free once exps are done
        for k in range(NK):
            nc.tensor.matmul(
                acc1[:],
                yTc[k][:, t * 128 : (t + 1) * 128],
                wpall[:, k * C + 512 : (k + 1) * C],
                start=(k == 0),
                stop=(k == NK - 1) and bp_d is None,
            )
        if bp_d is not None:
            nc.tensor.matmul(
                acc1[:], ones[0:1, 0:128], bp_row[0:1, 512:768],
                start=False, stop=True,
            )
        if t == NT - 1:
            nc.vector.tensor_copy(osb[:, 512:768], acc1[:])
        else:
            nc.scalar.copy(osb[:, 512:768], acc1[:])
        nc.sync.dma_start(
            y_d.ap()[t * 128 : (t + 1) * 128, 512:768], osb[:, 512:768]
        )

    # --- schedule ---
    # p-state warmup: the PE needs ~3us of continuous busy to reach max clock;
    # burn identity matmuls into the (otherwise idle) sc bank while the first
    # DMAs are in flight so real matmuls start at peak frequency.
    for i in range(41 if warmup else 0):
        wsc = pt([128, 1024], "sc") if i % 8 == 0 else wsc  # noqa: F821
        wid = ident32 if i < 3 else ident  # f32 rows are 4x slower: good early
        nc.tensor.matmul(
            wsc[:, (i % 8) * 128 : (i % 8 + 1) * 128],
            wid[:],
            wid[:],
            start=True,
            stop=True,
        )

    fill = Filler()
    # chunk 0 k-major: all four acc groups (qT0/kT0 x g0/g1) stream each x
    # chunk as it arrives; kT0 accs borrow the (still idle) av banks
    c0_accs = {}
    for k in range(NK):
        for dst, m, gg, tag in (
            (qT[0], 0, 0, "qkv"), (qT[0], 0, 1, "qkv"),
            (kT[0], 6, 0, "av"), (kT[0], 6, 1, "av"),
        ):
            key = (m, gg)
            if k == 0:
                c0_accs[key] = pt([128, 512], tag)
            nc.tensor.matmul(
                c0_accs[key][:],
                wa_of[m][:, k * 128 : (k + 1) * 128],
                xT[k][:, gg * 512 : (gg + 1) * 512],
                start=(k == 0),
                stop=(k == NK - 1),
            )
    for dst, m, gg in (
        (qT[0], 0, 0), (qT[0], 0, 1), (kT[0], 6, 0), (kT[0], 6, 1)
    ):
        dst_ap = dst[:, gg * 512 : (gg + 1) * 512]
        if ba_d is not None:
            nc.vector.tensor_scalar_add(dst_ap, c0_accs[(m, gg)][:], bqk[m][:])
        else:
            nc.scalar.copy(dst_ap, c0_accs[(m, gg)][:])
    emit_sc(0)
    for th in qk_unit_thunks(qT[1], 1):
        th()
    emit_sc(1)
    for th in qk_unit_thunks(kT[1], 7):
        th()
    emit_v(range(0, NT))
    # q/k of same chunk adjacent: qT2,kT2,qT3,kT3,...
    units = [u for c in range(2, NK) for u in ((qT[c], c), (kT[c], c + 6))]
    for h in range(2, H):
        if h >= 4:  # chunk h//2 must be fully emitted before its scores
            fill.ensure(2 * (h // 2 - 2) + 1)
        emit_sc(h)
        if h - 2 < len(units):
            fill.add(qk_unit_thunks(*units[h - 2]), uid=h - 2)
        if h == 10:
            fill.add(tpdma_thunks([0, 1, 2, 3]))
        if h == 11:
            fill.add(tpdma_thunks([4]))
        emit_avT(h - 2, fill)
    # tail: avT(10) covered by proj partials; avT(11) finishes per tile
    fill.add(proj_partial_thunks(0, range(3)))
    fill.add(proj_partial_thunks(1, range(3)))
    emit_avT(10, fill)
    fill.drain()

    def tail_cb(t):
        tp_pe(5, t, tag="av")
        if t >= 1:
            proj_finish(t - 1)
            if t + 1 < NT:  # partials 0,1 are pre-queued before avT(10)
                fill.add(proj_partial_thunks(t + 1, range(3)))
                fill.drain()

    emit_avT(11, fill, cb=tail_cb)
    proj_finish(NT - 1)


# pool buffer counts, patched onto tile_pool via tags at tile() time
_SB_BUFS = {
    "ident": 1, "ident32": 1, "tri": 1, "tri32": 1, "ones": 1, "ones32": 1,
    "bqk": 12, "bv32": 1, "bvrow": 1, "bp32": 1, "bprow": 1,
    "xT": 6, "qkT": 12, "v": 8, "ynat": 8, "yT": 6,
    "wa": 12, "wv": 1, "wp": 1,
    "expt": 3, "rc": 4, "osb": 4,
}
_PS_BUFS = {"qkv": 2, "sc": 2, "av": 2}


_NC_CACHE = {}


def _get_nc(has_battn, has_bproj, num_devices=N_CORES):
    key = (has_battn, has_bproj, num_devices)
    if key not in _NC_CACHE:
        _NC_CACHE[key] = _build_nc(has_battn, has_bproj, num_devices)
    return _NC_CACHE[key]


class _CachedSpmdRunner:
    """Jit the 8-core shard_map once; re-execute on every kernel() call.

    Mirrors concourse.bass2jax.run_bass_via_pjrt's multi-core path, minus the
    per-call retrace/recompile.
    """

    def __init__(self, nc, n_cores):
        import jax
        from jax.sharding import Mesh, PartitionSpec
        from jax.experimental.shard_map import shard_map
        from concourse import bass2jax

        bass2jax.install_neuronx_cc_hook()
        assert nc.dbg_addr is None
        self.n_cores = n_cores
        partition_name = (
            nc.partition_id_tensor.name if nc.partition_id_tensor else None
        )
        in_names, out_names, out_avals = [], [], []
        for alloc in nc.m.functions[0].allocations:
            if not isinstance(alloc, mybir.MemoryLocationSet):
                continue
            name = alloc.memorylocations[0].name
            if alloc.kind == "ExternalInput":
                if name != partition_name:
                    in_names.append(name)
            elif alloc.kind == "ExternalOutput":
                out_names.append(name)
                out_avals.append(
                    jax.core.ShapedArray(
                        tuple(alloc.tensor_shape), mybir.dt.np(alloc.dtype)
                    )
                )
        self.in_names, self.out_names, self.out_avals = (
            in_names, out_names, out_avals,
        )
        all_in_names = list(in_names) + out_names
        if partition_name is not None:
            all_in_names.append(partition_name)

        def _body(*args):
            operands = list(args)
            if partition_name is not None:
                operands.append(bass2jax.partition_id_tensor())
            return tuple(
                bass2jax._bass_exec_p.bind(
                    *operands,
                    out_avals=tuple(out_avals),
                    in_names=tuple(all_in_names),
                    out_names=tuple(out_names),
                    lowering_input_output_aliases=(),
                    sim_require_finite=True,
                    sim_require_nnan=True,
                    nc=nc,
                )
            )

        import numpy as _np

        devices = jax.devices()[:n_cores]
        mesh = Mesh(_np.asarray(devices), ("core",))
        n_io = len(in_names) + len(out_names)
        self.fn = jax.jit(
            shard_map(
                _body,
                mesh=mesh,
                in_specs=(PartitionSpec("core"),) * n_io,
                out_specs=(PartitionSpec("core"),) * len(out_names),
                check_rep=False,
            ),
            keep_unused=True,
        )
        self.sharding = jax.sharding.NamedSharding(mesh, PartitionSpec("core"))

    def run(self, in_maps):
        import jax

        args = []
        for name in self.in_names:
            cat = np.concatenate(
                [np.asarray(m[name]) for m in in_maps], axis=0
            )
            args.append(jax.device_put(cat, self.sharding))
        for av in self.out_avals:
            z = np.zeros((self.n_cores * av.shape[0], *av.shape[1:]), av.dtype)
            args.append(jax.device_put(z, self.sharding))
        outs = self.fn(*args)
        return [
            {
                name: np.asarray(outs[i]).reshape(
                    self.n_cores, *self.out_avals[i].shape
                )[c]
                for i, name in enumerate(self.out_names)
            }
            for c in range(self.n_cores)
        ]


_RUNNER_CACHE = {}


def _bf16(a):
    import ml_dtypes

    return np.ascontiguousarray(np.asarray(a).astype(ml_dtypes.bfloat16))


def kernel(x, W_attn, b_attn, W_proj, b_proj):
    x = _bf16(np.asarray(x).transpose(0, 2, 1))  # [B, C, T]: device wants x^T
    W_attn = _bf16(W_attn)
    W_proj = _bf16(W_proj)
    b_attn = np.asarray(b_attn, dtype=np.float32)
    b_proj = np.asarray(b_proj, dtype=np.float32)
    has_battn = bool(np.any(b_attn != 0.0))
    has_bproj = bool(np.any(b_proj != 0.0))

    nc = _get_nc(has_battn, has_bproj)
    in_maps = []
    for b in range(N_CORES):
        m = {"x": x[b], "W_attn": W_attn, "W_proj": W_proj}
        if has_battn:
            m["b_attn"] = b_attn
        if has_bproj:
            m["b_proj"] = b_proj
        in_maps.append(m)

    key = (has_battn, has_bproj)
    try:
        if key not in _RUNNER_CACHE:
            _RUNNER_CACHE[key] = _CachedSpmdRunner(nc, N_CORES)
        results = _RUNNER_CACHE[key].run(in_maps)
    except Exception:
        results = run_bass_kernel_spmd(
            nc, in_maps, core_ids=list(range(N_CORES))
        ).results
    return np.stack([results[b]["y"] for b in range(N_CORES)]).astype(np.float32)


# revision 89
# speedup vs baseline: 1.3881x; 1.0311x over previous
"""Trainium2 Bass kernel for causal multi-head attention block.

Module: qkv = x @ W_attn + b_attn; causal softmax((q k^T)/sqrt(C)); y = (attn @ v) @ W_proj + b_proj
Shapes (hardcoded): x [8, 1024, 768], W_attn [768, 2304], W_proj [768, 768], H=12, D=64.

Sharding: data parallel over batch - core b computes batch element b (B == n_cores == 8).
No collectives; host scatters x and gathers y. Host pre-casts x/W to bf16.

Per-core dataflow (all matmul operands bf16, fp32 PSUM accumulate):
  1. x arrives host-pre-transposed as x^T [768,1024] bf16; 6 plain loads
  2. QT,KT [c,1024] = (W_attn chunk).T @ xT   (W stationary, xT moving)
     V natural [tok, 12*(64+1)]: per head [V_h | 1.0] so the AV matmul emits
     the softmax denominator as the 65th output column.
  3. per head h, blocks (query-group g, key-tile j) packed first-fit-decreasing
     into exactly-full 512-word PSUM bins, one bank per scores tile (sc x3 /
     av x3 ring depths beat wider tiles with fewer exps):
       scoresT[tk,tq] = KT_h.T @ QT_h   (causal-tight tiles only)
       e = Exp(scoresT/sqrt(768))       (ScalarE, PSUM->SBUF bf16)
       diag tiles *= upper-tri mask     (Pool)
       avT[tq, 65] += e_block.T @ [V_h|1]  (65 moving rows/tile-pair: ~2x
                                            fewer PE rows than 512-wide AV)
       Y[t, h*64:] = avT[:,0:64] * recip(avT[:,64])  (normalization fused
                                            into the PSUM->SBUF copy, DVE)
     Scores run two heads ahead of AV; QK-chunk matmuls interleave as PE
     filler so exp latency and fin pacing never stall the PE.
  4. Y -> yT via PE transposes; out[tq, c] = yT.T @ W_proj (+bias), DMA out.
"""

import math
from collections import deque

import numpy as np

import concourse.bass as bass
import concourse.mybir as mybir
from concourse import bacc, tile
from concourse.bass_utils import run_bass_kernel_spmd
from concourse.masks import make_identity, make_upper_triangular

N_CORES = 8
B, T, C = 8, 1024, 768
H, D = 12, 64
C3 = 3 * C
NT = T // 128          # 8 token tiles
NK = C // 128          # 6 contraction chunks
NG = T // 512          # 2 query groups of 512
SCALE = 1.0 / math.sqrt(C)

F32 = mybir.dt.float32
BF16 = mybir.dt.bfloat16
MULT = mybir.AluOpType.mult
EXP = mybir.ActivationFunctionType.Exp


def _head_plan():
    """Causal score blocks for one head, packed into PSUM tiles.

    Returns (tiles, ecol): tiles is a list of (blocks, tile_width) where each
    block is (g, j, nw, tq0, diag, sc_off, e_off); ecol[(t, j)] gives the
    column in the head's e tile holding scoresT[:, tq-tile t] for key-tile j.
    """
    blocks = []
    for g in range(NG):
        for j in range(4 * g + 4):
            nw = 512 - max(j * 128 - g * 512, 0)
            tq0 = g * 512 + (512 - nw)
            blocks.append([g, j, nw, tq0])
    # first-fit-decreasing into 512-wide bins (bins end up exactly full)
    blocks.sort(key=lambda b: -b[2])
    bins = []
    for b in blocks:
        for bn in bins:
            if sum(x[2] for x in bn) + b[2] <= 512:
                bn.append(b)
                break
        else:
            bins.append([b])
    # one bin per [128,512] PSUM tile; assign sc/e offsets
    tiles = []
    e_base = 0
    for bn in bins:
        tb, off = [], 0
        for g, j, nw, tq0 in bn:
            diag = j * 128 == tq0
            tb.append((g, j, nw, tq0, diag, off, e_base + off))
            off += nw
        tiles.append((tb, off))
        e_base += off
    assert e_base == 4608
    ecol = {}
    for tb, _ in tiles:
        for g, j, nw, tq0, diag, sc_off, e_off in tb:
            for t in range(4 * g, 4 * g + 4):
                lo = t * 128
                if tq0 <= lo and lo + 128 <= tq0 + nw:
                    ecol[(t, j)] = e_off + lo - tq0
    for t in range(NT):
        for j in range(t + 1):
            assert (t, j) in ecol, (t, j)
    return tiles, ecol


_PLAN_TILES, _PLAN_ECOL = _head_plan()


def _build_nc(has_battn: bool, has_bproj: bool, num_devices: int, repeat: int = 1):
    nc = bacc.Bacc(
        "TRN2", target_bir_lowering=False, debug=False, num_devices=num_devices
    )

    x_d = nc.dram_tensor("x", (C, T), BF16, kind="ExternalInput")  # pre-transposed on host
    wa_d = nc.dram_tensor("W_attn", (C, C3), BF16, kind="ExternalInput")
    wp_d = nc.dram_tensor("W_proj", (C, C), BF16, kind="ExternalInput")
    ba_d = bp_d = None
    if has_battn:
        ba_d = nc.dram_tensor("b_attn", (C3,), F32, kind="ExternalInput")
    if has_bproj:
        bp_d = nc.dram_tensor("b_proj", (C,), F32, kind="ExternalInput")
    y_d = nc.dram_tensor("y", (T, C), F32, kind="ExternalOutput")

    with tile.TileContext(nc) as tc:
        with (
            tc.tile_pool(name="sb", bufs=1) as sb,
            tc.tile_pool(name="ps", bufs=1, space="PSUM") as ps,
        ):
            for r in range(repeat):
                _emit(nc, sb, ps, x_d, wa_d, wp_d, ba_d, bp_d, y_d,
                      warmup=(r == 0))
    nc.compile()
    return nc


def _emit(nc, sb, ps, x_d, wa_d, wp_d, ba_d, bp_d, y_d, warmup=True):
    def st(shape, tag, dtype=BF16, name=None):
        return sb.tile(list(shape), dtype, tag=tag, bufs=_SB_BUFS[tag],
                       name=name or tag)

    def pt(shape, tag, dtype=F32, name=None):
        return ps.tile(list(shape), dtype, tag=tag, bufs=_PS_BUFS[tag],
                       name=name or tag)

    # --- constants ---
    ident32 = st([128, 128], "ident32", dtype=F32)
    make_identity(nc, ident32[:])
    ident = st([128, 128], "ident")
    nc.vector.tensor_copy(ident[:], ident32[:])
    tri32 = st([128, 128], "tri32", dtype=F32)
    make_upper_triangular(nc, tri32[:], val=1.0, diag=True)  # tri[r,c]=1 iff c>=r
    tri = st([128, 128], "tri")
    nc.vector.tensor_copy(tri[:], tri32[:])

    bqk = []
    ones = bv_row = bp_row = None
    if ba_d is not None or bp_d is not None:
        ones32 = st([1, 128], "ones32", dtype=F32)
        nc.gpsimd.memset(ones32[:], 1.0)
        ones = st([1, 128], "ones")
        nc.vector.tensor_copy(ones[:], ones32[:])
    if ba_d is not None:
        for m in range(12):  # Q,K outchan tiles 0..1535
            bt = st([128, 1], "bqk", dtype=F32)
            nc.sync.dma_start(bt[:], ba_d.ap()[m * 128 : (m + 1) * 128])
            bqk.append(bt)
        bv32 = st([1, C], "bv32", dtype=F32)
        nc.sync.dma_start(bv32[:], ba_d.ap()[2 * C : 3 * C])
        bv_row = st([1, C], "bvrow")
        nc.vector.tensor_copy(bv_row[:], bv32[:])
    if bp_d is not None:
        bp32 = st([1, C], "bp32", dtype=F32)
        nc.sync.dma_start(bp32[:], bp_d.ap())
        bp_row = st([1, C], "bprow")
        nc.vector.tensor_copy(bp_row[:], bp32[:])

    # --- persistent activations ---
    qT = [st([128, T], "qkT", name=f"qT{c}") for c in range(NK)]
    kT = [st([128, T], "qkT", name=f"kT{c}") for c in range(NK)]
    vt = [st([128, H * 65], "v", name=f"v{t}") for t in range(NT)]
    Yn = [st([128, C], "ynat", name=f"Y{t}") for t in range(NT)]
    yTc = [st([128, T], "yT", name=f"yT{c}") for c in range(NK)]

    # --- DMAs (SP queue; emission order = issue order) ---
    was = []

    def load_wa(m):
        wam = st([128, NK * 128], "wa", name=f"wa{m}")
        nc.sync.dma_start(
            wam[:],
            wa_d.ap()[:, m * 128 : (m + 1) * 128].rearrange(
                "(k p) n -> p k n", p=128
            ),
        )
        return wam

    was.append(load_wa(0))      # Q chunk 0
    was.append(load_wa(6))      # K chunk 0
    xT = []
    for c in range(NK):
        xt = st([128, T], "xT", name=f"xT{c}")
        xT.append(xt)
        nc.sync.dma_start(xt[:], x_d.ap()[c * 128 : (c + 1) * 128, :])
    was.append(load_wa(1))
    was.append(load_wa(7))
    # all W_attn V columns, every k-chunk, in one 3-D-AP DMA
    wvall = st([128, NK * C], "wv")
    nc.sync.dma_start(
        wvall[:],
        wa_d.ap()[:, 2 * C : 3 * C].rearrange("(k p) n -> p k n", p=128),
    )
    for c in range(2, NK):
        was.append(load_wa(c))
        was.append(load_wa(c + 6))
    wa_of = {}
    order = [0, 6, 1, 7] + [m for c in range(2, NK) for m in (c, c + 6)]
    for i, m in enumerate(order):
        wa_of[m] = was[i]
    wpall = st([128, NK * C], "wp")
    nc.sync.dma_start(
        wpall[:], wp_d.ap().rearrange("(k p) n -> p k n", p=128)
    )

    # --- QK-chunk unit: thunk list (filler granularity = 1 matmul) ---
    def qk_unit_thunks(dst, m, cp_eng=None):
        ths = []
        for g in range(NG):
            box = {}

            def mk(k, g=g, box=box):
                def run():
                    if k == 0:
                        box["acc"] = pt([128, 512], "qkv")
                    nc.tensor.matmul(
                        box["acc"][:],
                        wa_of[m][:, k * 128 : (k + 1) * 128],
                        xT[k][:, g * 512 : (g + 1) * 512],
                        start=(k == 0),
                        stop=(k == NK - 1),
                    )
                return run

            ths += [mk(k) for k in range(NK)]

            def cp(g=g, box=box, m=m):
                dst_ap = dst[:, g * 512 : (g + 1) * 512]
                if ba_d is not None:
                    nc.vector.tensor_scalar_add(dst_ap, box["acc"][:], bqk[m][:])
                elif cp_eng == "act":
                    nc.scalar.copy(dst_ap, box["acc"][:])
                else:
                    nc.vector.tensor_copy(dst_ap, box["acc"][:])

            ths.append(cp)
        return ths

    # --- V natural [tok, 12*(64+1)] ---
    def emit_v(trange):
        for t in trange:
            nc.gpsimd.memset(vt[t][:], 1.0)  # ones cols; data cols overwritten
            for co, w in ((0, 512), (512, 256)):
                acc = pt([128, w], "qkv")
                for k in range(NK):
                    nc.tensor.matmul(
                        acc[:],
                        xT[k][:, t * 128 : (t + 1) * 128],
                        wvall[:, k * C + co : k * C + co + w],
                        start=(k == 0),
                        stop=(k == NK - 1) and ba_d is None,
                    )
                if ba_d is not None:
                    nc.tensor.matmul(
                        acc[:],
                        ones[0:1, 0:128],
                        bv_row[0:1, co : co + w],
                        start=False,
                        stop=True,
                    )
                nc.vector.tensor_copy(
                    vt[t][:].rearrange("p (h e) -> p h e", e=65)[
                        :, co // D : (co + w) // D, 0:D
                    ],
                    acc[:].rearrange("p (h e) -> p h e", e=D),
                )

    # --- per-head scores + exp + mask ---
    e_of = {}

    def emit_sc(h):
        ch, off = h // 2, (h % 2) * 64
        e = st([128, 4608], "expt", name=f"e{h}")
        e_of[h] = e
        for tb, tw in _PLAN_TILES:
            sc = pt([128, 512], "sc")
            for g, j, nw, tq0, diag, sc_off, e_off in tb:
                nc.tensor.matmul(
                    sc[:, sc_off : sc_off + nw],
                    kT[ch][off : off + 64, j * 128 : (j + 1) * 128],
                    qT[ch][off : off + 64, tq0 : tq0 + nw],
                    start=True,
                    stop=True,
                )
            e_base = tb[0][6]
            nc.scalar.activation(
                e[:, e_base : e_base + tw], sc[:, 0:tw], EXP, scale=SCALE
            )
            for g, j, nw, tq0, diag, sc_off, e_off in tb:
                if diag:
                    nc.gpsimd.tensor_tensor(
                        e[:, e_off : e_off + 128],
                        e[:, e_off : e_off + 128],
                        tri[:],
                        op=MULT,
                    )

    # --- per-head AV^T + fused-normalization fin, interleaved with filler ---
    def tp_pe(chh, t, tag="qkv"):
        ptt = pt([128, 128], tag, dtype=BF16)
        nc.tensor.transpose(
            ptt[:], Yn[t][:, chh * 128 : (chh + 1) * 128], ident[:]
        )
        nc.vector.tensor_copy(yTc[chh][:, t * 128 : (t + 1) * 128], ptt[:])

    def _avT_tile(h, t, e):
        av = pt([128, 65], "av")
        for j in range(t + 1):
            nc.tensor.matmul(
                av[:],
                e[:, _PLAN_ECOL[(t, j)] : _PLAN_ECOL[(t, j)] + 128],
                vt[j][:, h * 65 : (h + 1) * 65],
                start=(j == 0),
                stop=(j == t),
            )
        rc = st([128, 1], "rc", dtype=F32)
        nc.vector.reciprocal(rc[:], av[:, 64:65])
        nc.vector.tensor_scalar_mul(
            Yn[t][:, h * D : (h + 1) * D], av[:, 0:D], rc[:]
        )

    def emit_avT(h, filler, inline_tp=None, cb=None):
        e = e_of.pop(h)
        for t in range(NT):
            _avT_tile(h, t, e)
            if inline_tp is not None:
                tp_pe(inline_tp, t)
            if cb is not None:
                cb(t)
            filler.step(2 if t < 5 else 1)

    def emit_avT_pair(ha, hb, filler, inline_tp=(None, None)):
        ea, eb = e_of.pop(ha), e_of.pop(hb)
        for t in range(NT):
            _avT_tile(ha, t, ea)
            if inline_tp[0] is not None:
                tp_pe(inline_tp[0], t)
            filler.step(2 if t < 5 else 1)
            _avT_tile(hb, t, eb)
            if inline_tp[1] is not None:
                tp_pe(inline_tp[1], t)
            filler.step(2 if t < 5 else 1)

    class Filler:
        def __init__(self):
            self.q = deque()

        def add(self, ths, uid=10**9):
            self.q.extend((uid, th) for th in ths)

        def step(self, n=1):
            for _ in range(n):
                if self.q:
                    self.q.popleft()[1]()

        def ensure(self, uid):
            while self.q and self.q[0][0] <= uid:
                self.q.popleft()[1]()

        def drain(self):
            while self.q:
                self.q.popleft()[1]()

    # --- Y -> yT transpose via DMA XBAR (zero-PE thunks for early chunks) ---
    def tpdma_thunks(chs):
        ths = []
        for chh in chs:
            for t in range(NT):
                def run(chh=chh, t=t):
                    nc.sync.dma_start_transpose(
                        yTc[chh][:, t * 128 : (t + 1) * 128],
                        Yn[t][:, chh * 128 : (chh + 1) * 128],
                    )
                ths.append(run)
        return ths

    # --- output projection, split so proj fills the attention tail:
    # proj_partial(t) = co0 contraction k0..4 (needs only chunks 0..4);
    # proj_finish(t) = co0 k5 + full co1 + copies + stores (needs chunk 5,
    # i.e. fin(h10/h11, t) + the inline ch5 transpose).
    pacc = {}

    def proj_partial_thunks(t, ks=range(5)):
        ths = []

        def mk(k, t=t):
            def run():
                if k == 0:
                    pacc[t] = pt([128, 512], "qkv")
                nc.tensor.matmul(
                    pacc[t][:],
                    yTc[k][:, t * 128 : (t + 1) * 128],
                    wpall[:, k * C : k * C + 512],
                    start=(k == 0),
                    stop=False,
                )
            return run

        return [mk(k) for k in ks]

    pacc1 = {}

    def proj_partial1_thunks(t, ks=range(3)):
        def mk(k, t=t):
            def run():
                if k == 0:
                    pacc1[t] = pt([128, 256], "sc")
                nc.tensor.matmul(
                    pacc1[t][:],
                    yTc[k][:, t * 128 : (t + 1) * 128],
                    wpall[:, k * C + 512 : (k + 1) * C],
                    start=(k == 0),
                    stop=False,
                )
            return run

        return [mk(k) for k in ks]

    def proj_finish(t):
        osb = st([128, C], "osb", dtype=F32)
        acc = pacc.pop(t)
        for k in (3, 4, 5):  # k3/k4 here so partials never wait late yT chunks
            nc.tensor.matmul(
                acc[:],
                yTc[k][:, t * 128 : (t + 1) * 128],
                wpall[:, k * C : k * C + 512],
                start=False,
                stop=(k == 5) and bp_d is None,
            )
        if bp_d is not None:
            nc.tensor.matmul(
                acc[:], ones[0:1, 0:128], bp_row[0:1, 0:512],
                start=False, stop=True,
            )
        nc.scalar.copy(osb[:, 0:512], acc[:])
        # store the big half as soon as its copy lands: only the small co1
        # strip transfers after the last compute
        nc.sync.dma_start(y_d.ap()[t * 128 : (t + 1) * 128, 0:512], osb[:, 0:512])
        acc1 = pt([128, 256], "sc")  # sc banks are free once exps are done
        for k in range(NK):
            nc.tensor.matmul(
                acc1[:],
                yTc[k][:, t * 128 : (t + 1) * 128],
                wpall[:, k * C + 512 : (k + 1) * C],
                start=(k == 0),
                stop=(k == NK - 1) and bp_d is None,
            )
        if bp_d is not None:
            nc.tensor.matmul(
                acc1[:], ones[0:1, 0:128], bp_row[0:1, 512:768],
                start=False, stop=True,
            )
        if t == NT - 1:
            nc.vector.tensor_copy(osb[:, 512:768], acc1[:])
        else:
            nc.scalar.copy(osb[:, 512:768], acc1[:])
        nc.sync.dma_start(
            y_d.ap()[t * 128 : (t + 1) * 128, 512:768], osb[:, 512:768]
        )

    # --- schedule ---
    # p-state warmup: the PE needs ~3us of continuous busy to reach max clock;
    # burn identity matmuls into the (otherwise idle) sc bank while the first
    # DMAs are in flight so real matmuls start at peak frequency.
    for i in range(41 if warmup else 0):
        wsc = pt([128, 512], "sc") if i % 4 == 0 else wsc  # noqa: F821
        wid = ident32 if i < 3 else ident  # f32 rows are 4x slower: good early
        nc.tensor.matmul(
            wsc[:, (i % 4) * 128 : (i % 4 + 1) * 128],
            wid[:],
            wid[:],
            start=True,
            stop=True,
        )

    fill = Filler()
    # chunk 0 k-major: all four acc groups (qT0/kT0 x g0/g1) stream each x
    # chunk as it arrives; kT0 accs borrow the (still idle) av banks
    c0_accs = {}
    for k in range(NK):
        for dst, m, gg, tag in (
            (qT[0], 0, 0, "qkv"), (qT[0], 0, 1, "qkv"),
            (kT[0], 6, 0, "av"), (kT[0], 6, 1, "av"),
        ):
            key = (m, gg)
            if k == 0:
                c0_accs[key] = pt([128, 512], tag)
            nc.tensor.matmul(
                c0_accs[key][:],
                wa_of[m][:, k * 128 : (k + 1) * 128],
                xT[k][:, gg * 512 : (gg + 1) * 512],
                start=(k == 0),
                stop=(k == NK - 1),
            )
    for dst, m, gg in (
        (qT[0], 0, 0), (qT[0], 0, 1), (kT[0], 6, 0), (kT[0], 6, 1)
    ):
        dst_ap = dst[:, gg * 512 : (gg + 1) * 512]
        if ba_d is not None:
            nc.vector.tensor_scalar_add(dst_ap, c0_accs[(m, gg)][:], bqk[m][:])
        else:
            nc.scalar.copy(dst_ap, c0_accs[(m, gg)][:])
    emit_sc(0)
    for th in qk_unit_thunks(qT[1], 1):
        th()
    emit_sc(1)
    for th in qk_unit_thunks(kT[1], 7):
        th()
    emit_v(range(0, NT))
    # q/k of same chunk adjacent: qT2,kT2,qT3,kT3,...
    units = [u for c in range(2, NK) for u in ((qT[c], c), (kT[c], c + 6))]
    for h in range(2, H):
        if h >= 4:  # chunk h//2 must be fully emitted before its scores
            fill.ensure(2 * (h // 2 - 2) + 1)
        emit_sc(h)
        if h - 2 < len(units):
            fill.add(qk_unit_thunks(*units[h - 2]), uid=h - 2)
        if h == 10:
            fill.add(tpdma_thunks([0, 1, 2, 3]))
        if h == 11:
            fill.add(tpdma_thunks([4]))
        emit_avT(h - 2, fill)
    # tail: avT(10) covered by proj partials; avT(11) finishes per tile
    fill.add(proj_partial_thunks(0, range(3)))
    fill.add(proj_partial_thunks(1, range(3)))
    emit_avT(10, fill)
    fill.drain()

    def tail_cb(t):
        tp_pe(5, t, tag="av")
        if t >= 1:
            proj_finish(t - 1)
            if t + 1 < NT:  # partials 0,1 are pre-queued before avT(10)
                fill.add(proj_partial_thunks(t + 1, range(3)))
                fill.drain()

    emit_avT(11, fill, cb=tail_cb)
    proj_finish(NT - 1)


# pool buffer counts, patched onto tile_pool via tags at tile() time
_SB_BUFS = {
    "ident": 1, "ident32": 1, "tri": 1, "tri32": 1, "ones": 1, "ones32": 1,
    "bqk": 12, "bv32": 1, "bvrow": 1, "bp32": 1, "bprow": 1,
    "xT": 6, "qkT": 12, "v": 8, "ynat": 8, "yT": 6,
    "wa": 12, "wv": 1, "wp": 1,
    "expt": 3, "rc": 4, "osb": 4,
}
_PS_BUFS = {"qkv": 2, "sc": 3, "av": 3}


_NC_CACHE = {}


def _get_nc(has_battn, has_bproj, num_devices=N_CORES):
    key = (has_battn, has_bproj, num_devices)
    if key not in _NC_CACHE:
        _NC_CACHE[key] = _build_nc(has_battn, has_bproj, num_devices)
    return _NC_CACHE[key]


class _CachedSpmdRunner:
    """Jit the 8-core shard_map once; re-execute on every kernel() call.

    Mirrors concourse.bass2jax.run_bass_via_pjrt's multi-core path, minus the
    per-call retrace/recompile.
    """

    def __init__(self, nc, n_cores):
        import jax
        from jax.sharding import Mesh, PartitionSpec
        from jax.experimental.shard_map import shard_map
        from concourse import bass2jax

        bass2jax.install_neuronx_cc_hook()
        assert nc.dbg_addr is None
        self.n_cores = n_cores
        partition_name = (
            nc.partition_id_tensor.name if nc.partition_id_tensor else None
        )
        in_names, out_names, out_avals = [], [], []
        for alloc in nc.m.functions[0].allocations:
            if not isinstance(alloc, mybir.MemoryLocationSet):
                continue
            name = alloc.memorylocations[0].name
            if alloc.kind == "ExternalInput":
                if name != partition_name:
                    in_names.append(name)
            elif alloc.kind == "ExternalOutput":
                out_names.append(name)
                out_avals.append(
                    jax.core.ShapedArray(
                        tuple(alloc.tensor_shape), mybir.dt.np(alloc.dtype)
                    )
                )
        self.in_names, self.out_names, self.out_avals = (
            in_names, out_names, out_avals,
        )
        all_in_names = list(in_names) + out_names
        if partition_name is not None:
            all_in_names.append(partition_name)

        def _body(*args):
            operands = list(args)
            if partition_name is not None:
                operands.append(bass2jax.partition_id_tensor())
            return tuple(
                bass2jax._bass_exec_p.bind(
                    *operands,
                    out_avals=tuple(out_avals),
                    in_names=tuple(all_in_names),
                    out_names=tuple(out_names),
                    lowering_input_output_aliases=(),
                    sim_require_finite=True,
                    sim_require_nnan=True,
                    nc=nc,
                )
            )

        import numpy as _np

        devices = jax.devices()[:n_cores]
        mesh = Mesh(_np.asarray(devices), ("core",))
        n_io = len(in_names) + len(out_names)
        self.fn = jax.jit(
            shard_map(
                _body,
                mesh=mesh,
                in_specs=(PartitionSpec("core"),) * n_io,
                out_specs=(PartitionSpec("core"),) * len(out_names),
                check_rep=False,
            ),
            keep_unused=True,
        )
        self.sharding = jax.sharding.NamedSharding(mesh, PartitionSpec("core"))

    def run(self, in_maps):
        import jax

        args = []
        for name in self.in_names:
            cat = np.concatenate(
                [np.asarray(m[name]) for m in in_maps], axis=0
            )
            args.append(jax.device_put(cat, self.sharding))
        for av in self.out_avals:
            z = np.zeros((self.n_cores * av.shape[0], *av.shape[1:]), av.dtype)
            args.append(jax.device_put(z, self.sharding))
        outs = self.fn(*args)
        return [
            {
                name: np.asarray(outs[i]).reshape(
                    self.n_cores, *self.out_avals[i].shape
                )[c]
                for i, name in enumerate(self.out_names)
            }
            for c in range(self.n_cores)
        ]


_RUNNER_CACHE = {}


def _bf16(a):
    import ml_dtypes

    return np.ascontiguousarray(np.asarray(a).astype(ml_dtypes.bfloat16))


def kernel(x, W_attn, b_attn, W_proj, b_proj):
    x = _bf16(np.asarray(x).transpose(0, 2, 1))  # [B, C, T]: device wants x^T
    W_attn = _bf16(W_attn)
    W_proj = _bf16(W_proj)
    b_attn = np.asarray(b_attn, dtype=np.float32)
    b_proj = np.asarray(b_proj, dtype=np.float32)
    has_battn = bool(np.any(b_attn != 0.0))
    has_bproj = bool(np.any(b_proj != 0.0))

    nc = _get_nc(has_battn, has_bproj)
    in_maps = []
    for b in range(N_CORES):
        m = {"x": x[b], "W_attn": W_attn, "W_proj": W_proj}
        if has_battn:
            m["b_attn"] = b_attn
        if has_bproj:
            m["b_proj"] = b_proj
        in_maps.append(m)

    key = (has_battn, has_bproj)
    try:
        if key not in _RUNNER_CACHE:
            _RUNNER_CACHE[key] = _CachedSpmdRunner(nc, N_CORES)
        results = _RUNNER_CACHE[key].run(in_maps)
    except Exception:
        results = run_bass_kernel_spmd(
            nc, in_maps, core_ids=list(range(N_CORES))
        ).results
    return np.stack([results[b]["y"] for b in range(N_CORES)]).astype(np.float32)
